# revision 42
# baseline (speedup 1.0000x reference)
"""Multi-head differential attention on 8 Trainium2 NeuronCores.

Sharding: data-parallel over batch (B=2) x tensor-parallel over heads
(16 heads -> 4 per core). Core c handles batch c//4 and heads
4*(c%4) .. 4*(c%4)+3. Each core computes its heads' attention output and a
partial output projection; the host sums the 4 partials per batch.

v3: bf16 matmul operands, map-packed Q/K (concurrent K=64 score matmuls),
software-pipelined attention emission (PV/SM staggered 2 chunks behind
scores/exp so the in-order PE queue never waits on the scalar engine),
lazy weight DMAs so the first matmul isn't stuck behind the full DMA
trigger queue, batched RMS sqrt (one ACT table switch instead of 32),
rope interleaved into the projection phases.
"""

import math
import os
import sys

sys.path.insert(0, "/opt/trn_rl_repo")

import numpy as np

B, S, HID, NH = 2, 2048, 2048, 16
HD = HID // NH          # 128
QKD = HD // 2           # 64
NCORES = 8
GRPS = NCORES // B      # head groups per batch
HPC = NH // GRPS        # heads per core = 4
LAYER_ID = 1
LAMBDA_INIT = 0.8 - 0.6 * math.exp(-0.3 * LAYER_ID)
EPS = 1e-6

NB = S // 512           # 4 seq blocks of 512
NKC = S // 128          # 16 key chunks of 128

_PROGRAM = None         # compiled bass program, reused across calls


def _build_program():
    import concourse.bass as bass
    import concourse.tile as tile
    from concourse import bacc, mybir

    f32 = mybir.dt.float32
    f32r = mybir.dt.float32r
    bf16 = mybir.dt.bfloat16
    Alu = mybir.AluOpType
    Act = mybir.ActivationFunctionType

    nc = bacc.Bacc(None, target_bir_lowering=False, debug=False)

    def din(name, shape, dt=bf16):
        return nc.dram_tensor(name, shape, dt, kind="ExternalInput").ap()

    io = {
        "xq_t": din("xq_t", [HID, S]),
        "xk_t": din("xk_t", [HID, S]),
        "xv_t": din("xv_t", [HID, S]),
        "wq_t": din("wq_t", [HID, 512]),
        "wk_t": din("wk_t", [HID, 512]),
        "wv_t": din("wv_t", [HID, 512]),
        "wo_t": din("wo_t", [512, HID]),
        "crep": din("crep", [128, S]),
        "srep": din("srep", [128, S]),
        "pmat": din("pmat", [128, 128]),
        "tri01": din("tri01", [128, 128]),
        "ones_a": din("ones_a", [128, 128]),
        "o64": din("o64", [128, 128]),
        "neglam": din("neglam", [128, 1], f32),
    }
    y_t = nc.dram_tensor("y_t", [HID, S], bf16, kind="ExternalOutput").ap()

    from contextlib import ExitStack

    with tile.TileContext(nc) as tc, ExitStack() as ctx:
        persist = ctx.enter_context(tc.tile_pool(name="persist", bufs=1))
        constp = ctx.enter_context(tc.tile_pool(name="constp", bufs=1))

        # constants
        crep = constp.tile([128, S], bf16, name="crep_sb", tag="crep")
        srep = constp.tile([128, S], bf16, name="srep_sb", tag="srep")
        pmat = constp.tile([128, 128], bf16, name="pmat_sb", tag="pmat")
        tri01 = constp.tile([128, 128], bf16, name="tri01_sb", tag="tri01")
        ones_a = constp.tile([128, 128], bf16, name="ones_a_sb", tag="ones_a")
        o64 = constp.tile([128, 128], bf16, name="o64_sb", tag="o64")
        neglam = constp.tile([128, 1], f32, name="neglam_sb", tag="neglam")
        epsb = constp.tile([128, 1], f32, name="epsb", tag="epsb")
        nc.vector.memset(epsb[:], EPS)

        def emit_const_dmas():
            # deferred onto the scalar engine's DGE queue (behind the wq
            # chunks): keeps 1MB of table transfers out of the sync queue so
            # neither the first matmul nor the x-chunk stream waits on them
            for t, key in ((crep, "crep"), (srep, "srep"), (pmat, "pmat"),
                           (tri01, "tri01"), (ones_a, "ones_a"),
                           (o64, "o64"), (neglam, "neglam")):
                nc.scalar.dma_start(out=t[:], in_=io[key][:])

        # persistent tensors, per head h: Q^T/K^T [128, S] bf16 where
        # partitions [64g, 64g+64) hold softmax-map g's features.
        QT = [persist.tile([128, S], bf16, name=f"qt{h}", tag=f"qt{h}")
              for h in range(HPC)]
        KT = [persist.tile([128, S], bf16, name=f"kt{h}", tag=f"kt{h}")
              for h in range(HPC)]
        # V natural layout per 128-seq chunk: [128 seq, 4 heads * 128 feat]
        VH = [persist.tile([128, 512], bf16, name=f"vh{s}", tag=f"vh{s}")
              for s in range(NKC)]
        # combined attention output, transposed: [feat, seq]
        U = [persist.tile([128, S], bf16, name=f"u{h}", tag=f"u{h}")
             for h in range(HPC)]
        # output projection weights + per-(h,qb) sum-of-squares staging
        wo = [persist.tile([128, S], bf16, name=f"wo{h}", tag=f"wo{h}")
              for h in range(HPC)]
        ssqb = [persist.tile([128, 512], bf16, name=f"ssqb{i}", tag=f"ssqb{i}")
                for i in range(HPC * NB)]

        # ------------- phase P + R: projections with rope interleaved -------
        # rope: y = x*cos + (P x)*sin, P = signed pair-swap (PE matmul).
        rope_q = []   # pending (tiles, n) rope tasks

        with tc.tile_pool(name="wp", bufs=1) as wp, \
             tc.tile_pool(name="xp", bufs=6) as xp, \
             tc.tile_pool(name="rtp", bufs=4) as rtp, \
             tc.tile_pool(name="pp", bufs=1, space="PSUM") as pp:

            def emit_rope_head():
                # one head's rope at a time, sprinkled between projection
                # kc-groups so the px psum reuse never stalls the PE queue
                T4, n, h = rope_q.pop(0)
                sl = slice(n * 512, (n + 1) * 512)
                px = pp.tile([128, 512], f32, name=f"px_{T4[h].name}_{n}",
                             tag="px", bufs=2)
                nc.tensor.matmul(px[:], pmat[:], T4[h][:, sl],
                                 start=True, stop=True)
                tmp = rtp.tile([128, 512], bf16,
                               name=f"rs_{T4[h].name}_{n}", tag="rs")
                nc.vector.tensor_mul(tmp[:], px[:], srep[:, sl])
                tmp2 = rtp.tile([128, 512], bf16,
                                name=f"rc_{T4[h].name}_{n}", tag="rc")
                nc.vector.tensor_mul(tmp2[:], T4[h][:, sl], crep[:, sl])
                nc.vector.tensor_add(T4[h][:, sl], tmp[:], tmp2[:])

            def rope_ready(blk_idx):
                if not rope_q:
                    return False
                T4, n, _h = rope_q[0]
                task_idx = n + (0 if T4 is QT else NB)
                return blk_idx - task_idx >= 2

            wnames = ("wq_t", "wk_t", "wv_t")
            wts = {m: [wp.tile([128, 512], bf16, name=f"{wnames[m]}_{kc}",
                               tag=f"w{m}_{kc}") for kc in range(NKC)]
                   for m in range(3)}

            def wdma(m, kc):
                # weight DMAs ride the scalar engine's DGE queue so they
                # never contend with the x-chunk triggers on sync
                nc.scalar.dma_start(out=wts[m][kc][:],
                                    in_=io[wnames[m]][kc * 128:(kc + 1) * 128, :])

            for mode, xname in enumerate(("xq_t", "xk_t", "xv_t")):
                wt = wts[mode]
                xin = io[xname]
                for n in range(NB):
                    blk_idx = mode * NB + n
                    ps = [pp.tile([128, 512], f32, name=f"pp{t}_{mode}_{n}",
                                  tag="pp", bufs=5) for t in range(4)]
                    for kc in range(NKC):
                        if kc % 4 == 3 and rope_ready(blk_idx):
                            emit_rope_head()
                        if mode == 0 and n == 0:
                            wdma(0, kc)       # lazy: wq arrives just ahead
                        xck = xp.tile([128, 512], bf16, name=f"x_{mode}_{n}_{kc}",
                                      tag="x")
                        nc.sync.dma_start(
                            out=xck[:],
                            in_=xin[kc * 128:(kc + 1) * 128,
                                    n * 512:(n + 1) * 512])
                        if mode == 0 and n == 1:
                            wdma(1, kc)       # wk during q-proj
                        elif mode == 0 and n == 2:
                            wdma(2, kc)       # wv during q-proj
                        elif mode == 0 and n == 3 and kc % 4 == 0:
                            h = kc // 4       # wo prefetch, needed in phase Y
                            nc.scalar.dma_start(
                                out=wo[h][:],
                                in_=io["wo_t"][h * 128:(h + 1) * 128, :])
                        for t in range(4):
                            if mode < 2:
                                lhsT = wt[kc][:, t * 128:(t + 1) * 128]
                                rhs = xck[:]
                            else:
                                lhsT = xck[:, t * 128:(t + 1) * 128]
                                rhs = wt[kc][:]
                            nc.tensor.matmul(ps[t][:], lhsT, rhs,
                                             start=(kc == 0), stop=(kc == NKC - 1))
                    # drains on the scalar engine (vector is busy with rope)
                    for t in range(4):
                        if mode == 0:
                            nc.scalar.copy(QT[t][:, n * 512:(n + 1) * 512],
                                           ps[t][:])
                        elif mode == 1:
                            nc.scalar.copy(KT[t][:, n * 512:(n + 1) * 512],
                                           ps[t][:])
                        else:
                            nc.scalar.copy(VH[n * 4 + t][:], ps[t][:])
                    if blk_idx == 0:
                        emit_const_dmas()
                    if mode == 0:
                        rope_q.extend((QT, n, h) for h in range(HPC))
                    elif mode == 1:
                        rope_q.extend((KT, n, h) for h in range(HPC))
            while rope_q:
                emit_rope_head()

        # ------------- phase A: attention, PV/SM staggered 2 chunks ---------
        with tc.tile_pool(name="spp", bufs=1, space="PSUM") as spp, \
             tc.tile_pool(name="pvp", bufs=1, space="PSUM") as pvp, \
             tc.tile_pool(name="smp", bufs=1, space="PSUM") as smp, \
             tc.tile_pool(name="ep", bufs=5) as ep, \
             tc.tile_pool(name="cb", bufs=2) as cb:
            rms_pending = None

            for h in range(HPC):
                for qb in range(NB):
                    pv = pvp.tile([128, 1024], f32, name=f"pv_{h}_{qb}",
                                  tag="pv")
                    sm = smp.tile([128, 1024], f32, name=f"sm_{h}_{qb}",
                                  tag="sm")
                    nkc = 4 * qb + 4
                    pvsm_pending = []

                    def make_pvsm(kc, E, qoff, first, last, h=h, pv=pv, sm=sm):
                        def emit():
                            for g in (0, 1):
                                nc.tensor.matmul(
                                    pv[:, g * 512 + qoff:g * 512 + 512],
                                    VH[kc][:, h * 128:(h + 1) * 128],
                                    E[:, g * 512 + qoff:g * 512 + 512],
                                    start=first, stop=last)
                            for g in (0, 1):
                                nc.tensor.matmul(
                                    sm[:, g * 512 + qoff:g * 512 + 512],
                                    ones_a[:],
                                    E[:, g * 512 + qoff:g * 512 + 512],
                                    start=first, stop=last)
                        return emit

                    for kc in range(nkc):
                        j = kc - 4 * qb  # >= 0 on the causal diagonal band
                        qoff = j * 128 if j >= 0 else 0
                        sp = spp.tile([128, 1024], f32, name=f"s_{h}_{qb}_{kc}",
                                      tag="sp", bufs=2)
                        for g in (0, 1):
                            goff = g * 512
                            nc.tensor.matmul(
                                sp[:, goff + qoff:goff + 512],
                                KT[h][g * 64:(g + 1) * 64,
                                      kc * 128:(kc + 1) * 128],
                                QT[h][g * 64:(g + 1) * 64,
                                      qb * 512 + qoff:(qb + 1) * 512],
                                start=True, stop=True)
                        E = ep.tile([128, 1024], bf16, name=f"e_{h}_{qb}_{kc}",
                                    tag="e")
                        if qoff == 0:
                            nc.scalar.activation(E[:], sp[:], Act.Exp,
                                                 scale=0.125)
                        else:
                            for g in (0, 1):
                                nc.scalar.activation(
                                    E[:, g * 512 + qoff:g * 512 + 512],
                                    sp[:, g * 512 + qoff:g * 512 + 512],
                                    Act.Exp, scale=0.125)
                        if j >= 0:
                            # causal mask inside the diagonal 128x128 block,
                            # applied multiplicatively on the vector engine
                            for g in (0, 1):
                                esl = E[:, g * 512 + qoff:g * 512 + qoff + 128]
                                nc.vector.tensor_mul(esl, esl, tri01[:])
                        pvsm_pending.append(make_pvsm(
                            kc, E, qoff, kc == 0, kc == nkc - 1))
                        if len(pvsm_pending) > 3:
                            pvsm_pending.pop(0)()
                        if kc == 2 and rms_pending is not None:
                            rms_pending()
                            rms_pending = None
                    while pvsm_pending:
                        pvsm_pending.pop(0)()

                    # combine: U = pv0/sm0 - lam * pv1/sm1
                    usl = U[h][:, qb * 512:(qb + 1) * 512]
                    rb = cb.tile([128, 1024], f32, name=f"rb_{h}_{qb}", tag="rb")
                    nc.vector.reciprocal_approx_fast(rb[:], sm[:])
                    t1 = cb.tile([128, 512], f32, name=f"t1_{h}_{qb}", tag="t1")
                    t2 = cb.tile([128, 512], f32, name=f"t2_{h}_{qb}", tag="t2")
                    nc.vector.tensor_mul(t1[:], pv[:, 0:512], rb[:, 0:512])
                    nc.vector.tensor_mul(t2[:], pv[:, 512:1024], rb[:, 512:1024])
                    nc.vector.scalar_tensor_tensor(
                        usl, t2[:], neglam[:], t1[:],
                        op0=Alu.mult, op1=Alu.add)
                    # reserve the sm-tag psum bank for this iteration's ssq
                    # before the next iteration's sm allocation
                    ssq = smp.tile([128, 512], f32, name=f"ssq_{h}_{qb}",
                                   tag="sm")

                    def make_rms(h=h, qb=qb, usl=usl, ssq=ssq):
                        def emit():
                            sq = cb.tile([128, 512], bf16, name=f"sq_{h}_{qb}",
                                         tag="sq")
                            nc.vector.tensor_mul(sq[:], usl, usl)
                            nc.tensor.matmul(ssq[:], ones_a[:], sq[:],
                                             start=True, stop=True)
                            nc.vector.tensor_copy(ssqb[h * NB + qb][:], ssq[:])
                        return emit
                    rms_pending = make_rms()
            if rms_pending is not None:
                rms_pending()
                rms_pending = None

        # ------------- phase N + Y: batched RMS norm, output projection -----
        with tc.tile_pool(name="yp", bufs=4, space="PSUM") as yp, \
             tc.tile_pool(name="ys", bufs=4) as ys, \
             tc.tile_pool(name="nb", bufs=4) as nb:
            def emit_norm(qc):
                for h in range(HPC):
                    usl = U[h][:, qc * 512:(qc + 1) * 512]
                    sd = nb.tile([128, 512], f32, name=f"sd_{h}_{qc}", tag="sd")
                    nc.scalar.activation(sd[:], ssqb[h * NB + qc][:], Act.Sqrt,
                                         scale=1.0 / HD, bias=epsb[:])
                    rstd = nb.tile([128, 512], f32, name=f"rstd_{h}_{qc}",
                                   tag="rstd")
                    nc.vector.reciprocal_approx_fast(rstd[:], sd[:])
                    nc.vector.tensor_mul(usl, usl, rstd[:])

            emit_norm(0)
            for qc in range(NB):
                # normalize for qc+1 queues ahead of Y(qc)'s psum drains so
                # the next column's U is ready before its first matmul
                if qc + 1 < NB:
                    emit_norm(qc + 1)
                for oc in range(NKC):
                    py = yp.tile([128, 512], f32, name=f"py_{oc}_{qc}", tag="py")
                    for h in range(HPC):
                        nc.tensor.matmul(
                            py[:],
                            wo[h][:, oc * 128:(oc + 1) * 128],
                            U[h][:, qc * 512:(qc + 1) * 512],
                            start=(h == 0), stop=(h == HPC - 1))
                    yst = ys.tile([128, 512], bf16, name=f"yst_{oc}_{qc}",
                                  tag="yst")
                    if (oc + qc) % 2 == 0:
                        nc.vector.tensor_copy(yst[:], py[:])
                    else:
                        nc.scalar.copy(yst[:], py[:])
                    nc.sync.dma_start(
                        out=y_t[oc * 128:(oc + 1) * 128, qc * 512:(qc + 1) * 512],
                        in_=yst[:])

    nc.compile()
    return nc


def _host_prep(q, k, v, Wq, Wk, Wv, Wo, lambda_q1, lambda_k1, lambda_q2,
               lambda_k2, gnorm_w, cos_emb, sin_emb):
    import ml_dtypes

    f32 = np.float32
    bf16 = ml_dtypes.bfloat16
    q = np.asarray(q, f32); k = np.asarray(k, f32); v = np.asarray(v, f32)
    Wq = np.asarray(Wq, f32); Wk = np.asarray(Wk, f32)
    Wv = np.asarray(Wv, f32); Wo = np.asarray(Wo, f32)
    gnorm_w = np.asarray(gnorm_w, f32)
    cos_emb = np.asarray(cos_emb, f32); sin_emb = np.asarray(sin_emb, f32)

    lam1 = np.exp(np.sum(np.asarray(lambda_q1, f32) * np.asarray(lambda_k1, f32),
                         dtype=f32))
    lam2 = np.exp(np.sum(np.asarray(lambda_q2, f32) * np.asarray(lambda_k2, f32),
                         dtype=f32))
    lam = np.float32(lam1 - lam2 + LAMBDA_INIT)

    # per-batch transposed activations (bf16)
    xt = {}
    for b in range(B):
        xt[("q", b)] = np.ascontiguousarray(q[b].T).astype(bf16)
        xt[("k", b)] = np.ascontiguousarray(k[b].T).astype(bf16)
        xt[("v", b)] = np.ascontiguousarray(v[b].T).astype(bf16)

    # shared constant tensors
    base_c = cos_emb[:S, :QKD]          # [S, 64]
    base_s = sin_emb[:S, :QKD]
    crep = np.ascontiguousarray(np.tile(base_c.T, (2, 1))).astype(bf16)  # [128, S]
    srep = np.ascontiguousarray(np.tile(base_s.T, (2, 1))).astype(bf16)
    pmat = np.zeros((128, 128), f32)
    for blk in range(2):
        o = blk * 64
        for i in range(QKD // 2):
            pmat[o + 2 * i, o + 2 * i + 1] = 1.0     # lhsT[2i, 2i+1]
            pmat[o + 2 * i + 1, o + 2 * i] = -1.0    # lhsT[2i+1, 2i]
    pmat = pmat.astype(bf16)
    # tri01[p, n] = 0 if p > n (key index > query index within the 128 block)
    tri01 = (np.triu(np.ones((128, 128), f32), 0)).astype(bf16)
    ones_a = np.ones((128, 128), bf16)
    o64 = np.full((128, 128), 1.0 / 64, bf16)
    neglam = np.full((128, 1), -lam, f32)

    per_core = []
    for c in range(NCORES):
        b, grp = c // GRPS, c % GRPS
        heads = [HPC * grp + j for j in range(HPC)]
        # wq/wk columns: tile h; partitions [64g, 64g+64) = map g of head h,
        # original feature order (interleaved pairs)
        cols = []
        for h in range(HPC):
            hg = heads[h]
            for g in range(2):
                cols.extend(hg * HD + g * QKD + d for d in range(QKD))
        cols = np.asarray(cols)
        vrows = np.asarray([h * HD + d for h in heads for d in range(HD)])
        wq_t = np.ascontiguousarray(Wq[cols, :].T).astype(bf16)
        wk_t = np.ascontiguousarray(Wk[cols, :].T).astype(bf16)
        wv_t = np.ascontiguousarray(Wv[vrows, :].T).astype(bf16)
        gtile = np.tile(gnorm_w, HPC)                       # [512]
        wo_t = np.ascontiguousarray(
            ((1.0 - LAMBDA_INIT) * Wo[:, vrows] * gtile[None, :]).T).astype(bf16)
        per_core.append({
            "xq_t": xt[("q", b)], "xk_t": xt[("k", b)], "xv_t": xt[("v", b)],
            "wq_t": wq_t, "wk_t": wk_t, "wv_t": wv_t, "wo_t": wo_t,
            "crep": crep, "srep": srep, "pmat": pmat,
            "tri01": tri01, "ones_a": ones_a, "o64": o64, "neglam": neglam,
        })
    return per_core


def _install_ntff_hook():
    """antenv.axon_hooks is absent in this image; synthesize it so
    run_bass_kernel_spmd(trace=True) can capture NTFF profiles."""
    import sys as _sys
    import types

    if "antenv.axon_hooks" in _sys.modules:
        return
    import antenv
    mod = types.ModuleType("antenv.axon_hooks")
    state = {"hook": None}
    mod.set_axon_ntff_profile_hook = lambda h: state.__setitem__("hook", h)
    mod.get_axon_ntff_profile_hook = lambda: state["hook"]
    _sys.modules["antenv.axon_hooks"] = mod
    antenv.axon_hooks = mod
    try:
        from trn_agent_boot.trn_boot import _ntff_profile_via_ctypes
        state["hook"] = _ntff_profile_via_ctypes("/opt/axon/libaxon_pjrt.so")
    except Exception as e:  # degrade: trace skipped, run still works
        print("ntff hook install failed:", e)


def kernel(q, k, v, Wq, Wk, Wv, Wo, lambda_q1, lambda_k1, lambda_q2,
           lambda_k2, gnorm_w, cos_emb, sin_emb, mask, _trace=False):
    if _trace:
        _install_ntff_hook()
    global _PROGRAM
    if _PROGRAM is None:
        _PROGRAM = _build_program()
    nc = _PROGRAM

    in_maps = _host_prep(q, k, v, Wq, Wk, Wv, Wo, lambda_q1, lambda_k1,
                         lambda_q2, lambda_k2, gnorm_w, cos_emb, sin_emb)

    from concourse.bass_utils import run_bass_kernel_spmd
    res = run_bass_kernel_spmd(nc, in_maps, core_ids=list(range(NCORES)),
                               trace=_trace)
    kernel.last_result = res

    y = np.zeros((B, S, HID), np.float32)
    for c in range(NCORES):
        y[c // GRPS] += np.asarray(res.results[c]["y_t"]).T.astype(np.float32)
    return y


# revision 45
# speedup vs baseline: 1.1339x; 1.1339x over previous
"""Multi-head differential attention on 8 Trainium2 NeuronCores.

Sharding: data-parallel over batch (B=2) x tensor-parallel over heads
(16 heads -> 4 per core). Core c handles batch c//4 and heads
4*(c%4) .. 4*(c%4)+3. Each core computes its heads' attention output and a
partial output projection; the host sums the 4 partials per batch.

v3: bf16 matmul operands, map-packed Q/K (concurrent K=64 score matmuls),
software-pipelined attention emission (PV/SM staggered 2 chunks behind
scores/exp so the in-order PE queue never waits on the scalar engine),
lazy weight DMAs so the first matmul isn't stuck behind the full DMA
trigger queue, batched RMS sqrt (one ACT table switch instead of 32),
rope interleaved into the projection phases.
"""

import math
import os
import sys

sys.path.insert(0, "/opt/trn_rl_repo")

import numpy as np

B, S, HID, NH = 2, 2048, 2048, 16
HD = HID // NH          # 128
QKD = HD // 2           # 64
NCORES = 8
GRPS = NCORES // B      # head groups per batch
HPC = NH // GRPS        # heads per core = 4
LAYER_ID = 1
LAMBDA_INIT = 0.8 - 0.6 * math.exp(-0.3 * LAYER_ID)
EPS = 1e-6

NB = S // 512           # 4 seq blocks of 512
NKC = S // 128          # 16 key chunks of 128

_PROGRAM = None         # compiled bass program, reused across calls


def _build_program():
    import concourse.bass as bass
    import concourse.tile as tile
    from concourse import bacc, mybir

    f32 = mybir.dt.float32
    f32r = mybir.dt.float32r
    bf16 = mybir.dt.bfloat16
    Alu = mybir.AluOpType
    Act = mybir.ActivationFunctionType

    nc = bacc.Bacc(None, target_bir_lowering=False, debug=False)

    def din(name, shape, dt=bf16):
        return nc.dram_tensor(name, shape, dt, kind="ExternalInput").ap()

    io = {
        "xq_t": din("xq_t", [HID, S]),
        "xk_t": din("xk_t", [HID, S]),
        "xv_t": din("xv_t", [HID, S]),
        "wq_t": din("wq_t", [HID, 512]),
        "wk_t": din("wk_t", [HID, 512]),
        "wv_t": din("wv_t", [HID, 512]),
        "wo_t": din("wo_t", [512, HID]),
        "crep": din("crep", [128, S]),
        "srep": din("srep", [128, S]),
        "pmat": din("pmat", [128, 128]),
        "tri01": din("tri01", [128, 128]),
        "ones_a": din("ones_a", [128, 128]),
        "o64": din("o64", [128, 128]),
        "neglam": din("neglam", [128, 1], f32),
    }
    y_t = nc.dram_tensor("y_t", [HID, S], bf16, kind="ExternalOutput").ap()

    from contextlib import ExitStack

    with tile.TileContext(nc) as tc, ExitStack() as ctx:
        persist = ctx.enter_context(tc.tile_pool(name="persist", bufs=1))
        constp = ctx.enter_context(tc.tile_pool(name="constp", bufs=1))

        # constants
        crep = constp.tile([128, S], bf16, name="crep_sb", tag="crep")
        srep = constp.tile([128, S], bf16, name="srep_sb", tag="srep")
        pmat = constp.tile([128, 128], bf16, name="pmat_sb", tag="pmat")
        tri01 = constp.tile([128, 128], bf16, name="tri01_sb", tag="tri01")
        ones_a = constp.tile([128, 128], bf16, name="ones_a_sb", tag="ones_a")
        o64 = constp.tile([128, 128], bf16, name="o64_sb", tag="o64")
        neglam = constp.tile([128, 1], f32, name="neglam_sb", tag="neglam")
        epsb = constp.tile([128, 1], f32, name="epsb", tag="epsb")
        nc.vector.memset(epsb[:], EPS)

        def emit_const_dmas():
            # deferred onto the scalar engine's DGE queue (behind the wq
            # chunks): keeps 1MB of table transfers out of the sync queue so
            # neither the first matmul nor the x-chunk stream waits on them
            for t, key in ((crep, "crep"), (srep, "srep"), (pmat, "pmat"),
                           (tri01, "tri01"), (ones_a, "ones_a"),
                           (o64, "o64"), (neglam, "neglam")):
                nc.scalar.dma_start(out=t[:], in_=io[key][:])

        # persistent tensors, per head h: Q^T/K^T [128, S] bf16 where
        # partitions [64g, 64g+64) hold softmax-map g's features.
        QT = [persist.tile([128, S], bf16, name=f"qt{h}", tag=f"qt{h}")
              for h in range(HPC)]
        KT = [persist.tile([128, S], bf16, name=f"kt{h}", tag=f"kt{h}")
              for h in range(HPC)]
        # V natural layout per 128-seq chunk: [128 seq, 4 heads * 128 feat]
        VH = [persist.tile([128, 512], bf16, name=f"vh{s}", tag=f"vh{s}")
              for s in range(NKC)]
        # combined attention output, transposed: [feat, seq]
        U = [persist.tile([128, S], bf16, name=f"u{h}", tag=f"u{h}")
             for h in range(HPC)]
        # output projection weights + per-(h,qb) sum-of-squares staging
        wo = [persist.tile([128, S], bf16, name=f"wo{h}", tag=f"wo{h}")
              for h in range(HPC)]
        ssqb = [persist.tile([128, 512], bf16, name=f"ssqb{i}", tag=f"ssqb{i}")
                for i in range(HPC * NB)]

        # ------------- phase P + R: projections with rope interleaved -------
        # rope: y = x*cos + (P x)*sin, P = signed pair-swap (PE matmul).
        rope_q = []   # pending (tiles, n) rope tasks

        with tc.tile_pool(name="wp", bufs=1) as wp, \
             tc.tile_pool(name="xp", bufs=6) as xp, \
             tc.tile_pool(name="rtp", bufs=4) as rtp, \
             tc.tile_pool(name="pp", bufs=1, space="PSUM") as pp:

            def emit_rope():
                T4, n = rope_q.pop(0)
                sl = slice(n * 512, (n + 1) * 512)
                pxs = []
                for h in range(HPC):
                    px = pp.tile([128, 512], f32, name=f"px_{T4[h].name}_{n}",
                                 tag="px", bufs=3)
                    nc.tensor.matmul(px[:], pmat[:], T4[h][:, sl],
                                     start=True, stop=True)
                    pxs.append(px)
                for h in range(HPC):
                    tmp = rtp.tile([128, 512], bf16,
                                   name=f"rs_{T4[h].name}_{n}", tag="rs")
                    nc.vector.tensor_mul(tmp[:], pxs[h][:], srep[:, sl])
                    tmp2 = rtp.tile([128, 512], bf16,
                                    name=f"rc_{T4[h].name}_{n}", tag="rc")
                    nc.vector.tensor_mul(tmp2[:], T4[h][:, sl], crep[:, sl])
                    nc.vector.tensor_add(T4[h][:, sl], tmp[:], tmp2[:])

            wnames = ("wq_t", "wk_t", "wv_t")
            wts = {m: [wp.tile([128, 512], bf16, name=f"{wnames[m]}_{kc}",
                               tag=f"w{m}_{kc}") for kc in range(NKC)]
                   for m in range(3)}

            def wdma(m, kc):
                # weight DMAs ride the scalar engine's DGE queue so they
                # never contend with the x-chunk triggers on sync
                nc.scalar.dma_start(out=wts[m][kc][:],
                                    in_=io[wnames[m]][kc * 128:(kc + 1) * 128, :])

            for mode, xname in enumerate(("xq_t", "xk_t", "xv_t")):
                wt = wts[mode]
                xin = io[xname]
                for n in range(NB):
                    blk_idx = mode * NB + n
                    ps = [pp.tile([128, 512], f32, name=f"pp{t}_{mode}_{n}",
                                  tag="pp", bufs=5) for t in range(4)]
                    for kc in range(NKC):
                        if mode == 0 and n == 0:
                            wdma(0, kc)       # lazy: wq arrives just ahead
                        xck = xp.tile([128, 512], bf16, name=f"x_{mode}_{n}_{kc}",
                                      tag="x")
                        nc.sync.dma_start(
                            out=xck[:],
                            in_=xin[kc * 128:(kc + 1) * 128,
                                    n * 512:(n + 1) * 512])
                        if mode == 0 and n == 1:
                            wdma(1, kc)       # wk during q-proj
                        elif mode == 0 and n == 2:
                            wdma(2, kc)       # wv during q-proj
                        elif mode == 0 and n == 3 and kc % 4 == 0:
                            h = kc // 4       # wo prefetch, needed in phase Y
                            nc.scalar.dma_start(
                                out=wo[h][:],
                                in_=io["wo_t"][h * 128:(h + 1) * 128, :])
                        for t in range(4):
                            if mode < 2:
                                lhsT = wt[kc][:, t * 128:(t + 1) * 128]
                                rhs = xck[:]
                            else:
                                lhsT = xck[:, t * 128:(t + 1) * 128]
                                rhs = wt[kc][:]
                            nc.tensor.matmul(ps[t][:], lhsT, rhs,
                                             start=(kc == 0), stop=(kc == NKC - 1))
                    # drains on the scalar engine (vector is busy with rope)
                    for t in range(4):
                        if mode == 0:
                            nc.scalar.copy(QT[t][:, n * 512:(n + 1) * 512],
                                           ps[t][:])
                        elif mode == 1:
                            nc.scalar.copy(KT[t][:, n * 512:(n + 1) * 512],
                                           ps[t][:])
                        else:
                            nc.scalar.copy(VH[n * 4 + t][:], ps[t][:])
                    if blk_idx == 0:
                        emit_const_dmas()
                    if mode == 0:
                        rope_q.append((QT, n))
                    elif mode == 1:
                        rope_q.append((KT, n))
                    if blk_idx >= 2 and rope_q and blk_idx - (
                            rope_q[0][1] + (0 if rope_q[0][0] is QT else NB)) >= 2:
                        emit_rope()
            while rope_q:
                emit_rope()

        # ------------- phase A: attention, PV/SM staggered 2 chunks ---------
        with tc.tile_pool(name="spp", bufs=1, space="PSUM") as spp, \
             tc.tile_pool(name="pvp", bufs=1, space="PSUM") as pvp, \
             tc.tile_pool(name="smp", bufs=1, space="PSUM") as smp, \
             tc.tile_pool(name="ep", bufs=5) as ep, \
             tc.tile_pool(name="cb", bufs=2) as cb:
            rms_pending = None

            for h in range(HPC):
                for qb in range(NB):
                    pv = pvp.tile([128, 1024], f32, name=f"pv_{h}_{qb}",
                                  tag="pv")
                    sm = smp.tile([128, 1024], f32, name=f"sm_{h}_{qb}",
                                  tag="sm")
                    nkc = 4 * qb + 4
                    pvsm_pending = []

                    def make_pvsm(kc, E, qoff, first, last, h=h, pv=pv, sm=sm):
                        def emit():
                            for g in (0, 1):
                                nc.tensor.matmul(
                                    pv[:, g * 512 + qoff:g * 512 + 512],
                                    VH[kc][:, h * 128:(h + 1) * 128],
                                    E[:, g * 512 + qoff:g * 512 + 512],
                                    start=first, stop=last)
                            for g in (0, 1):
                                nc.tensor.matmul(
                                    sm[:, g * 512 + qoff:g * 512 + 512],
                                    ones_a[:],
                                    E[:, g * 512 + qoff:g * 512 + 512],
                                    start=first, stop=last)
                        return emit

                    for kc in range(nkc):
                        j = kc - 4 * qb  # >= 0 on the causal diagonal band
                        qoff = j * 128 if j >= 0 else 0
                        sp = spp.tile([128, 1024], f32, name=f"s_{h}_{qb}_{kc}",
                                      tag="sp", bufs=2)
                        for g in (0, 1):
                            goff = g * 512
                            nc.tensor.matmul(
                                sp[:, goff + qoff:goff + 512],
                                KT[h][g * 64:(g + 1) * 64,
                                      kc * 128:(kc + 1) * 128],
                                QT[h][g * 64:(g + 1) * 64,
                                      qb * 512 + qoff:(qb + 1) * 512],
                                start=True, stop=True)
                        E = ep.tile([128, 1024], bf16, name=f"e_{h}_{qb}_{kc}",
                                    tag="e")
                        if qoff == 0:
                            nc.scalar.activation(E[:], sp[:], Act.Exp,
                                                 scale=0.125)
                        else:
                            for g in (0, 1):
                                nc.scalar.activation(
                                    E[:, g * 512 + qoff:g * 512 + 512],
                                    sp[:, g * 512 + qoff:g * 512 + 512],
                                    Act.Exp, scale=0.125)
                        if j >= 0:
                            # causal mask inside the diagonal 128x128 block,
                            # applied multiplicatively on the vector engine
                            for g in (0, 1):
                                esl = E[:, g * 512 + qoff:g * 512 + qoff + 128]
                                nc.vector.tensor_mul(esl, esl, tri01[:])
                        pvsm_pending.append(make_pvsm(
                            kc, E, qoff, kc == 0, kc == nkc - 1))
                        if len(pvsm_pending) > 3:
                            pvsm_pending.pop(0)()
                        if kc == 2 and rms_pending is not None:
                            rms_pending()
                            rms_pending = None
                    while pvsm_pending:
                        pvsm_pending.pop(0)()

                    # combine: U = pv0/sm0 - lam * pv1/sm1
                    usl = U[h][:, qb * 512:(qb + 1) * 512]
                    rb = cb.tile([128, 1024], f32, name=f"rb_{h}_{qb}", tag="rb")
                    nc.vector.reciprocal_approx_fast(rb[:], sm[:])
                    t1 = cb.tile([128, 512], f32, name=f"t1_{h}_{qb}", tag="t1")
                    t2 = cb.tile([128, 512], f32, name=f"t2_{h}_{qb}", tag="t2")
                    nc.vector.tensor_mul(t1[:], pv[:, 0:512], rb[:, 0:512])
                    nc.vector.tensor_mul(t2[:], pv[:, 512:1024], rb[:, 512:1024])
                    nc.vector.scalar_tensor_tensor(
                        usl, t2[:], neglam[:], t1[:],
                        op0=Alu.mult, op1=Alu.add)
                    # reserve the sm-tag psum bank for this iteration's ssq
                    # before the next iteration's sm allocation
                    ssq = smp.tile([128, 512], f32, name=f"ssq_{h}_{qb}",
                                   tag="sm")

                    def make_rms(h=h, qb=qb, usl=usl, ssq=ssq):
                        def emit():
                            sq = cb.tile([128, 512], bf16, name=f"sq_{h}_{qb}",
                                         tag="sq")
                            nc.vector.tensor_mul(sq[:], usl, usl)
                            nc.tensor.matmul(ssq[:], ones_a[:], sq[:],
                                             start=True, stop=True)
                            nc.vector.tensor_copy(ssqb[h * NB + qb][:], ssq[:])
                        return emit
                    rms_pending = make_rms()
            if rms_pending is not None:
                rms_pending()
                rms_pending = None

        # ------------- phase N + Y: batched RMS norm, output projection -----
        with tc.tile_pool(name="yp", bufs=4, space="PSUM") as yp, \
             tc.tile_pool(name="ys", bufs=4) as ys, \
             tc.tile_pool(name="nb", bufs=4) as nb:
            def emit_norm(qc):
                for h in range(HPC):
                    usl = U[h][:, qc * 512:(qc + 1) * 512]
                    sd = nb.tile([128, 512], f32, name=f"sd_{h}_{qc}", tag="sd")
                    nc.scalar.activation(sd[:], ssqb[h * NB + qc][:], Act.Sqrt,
                                         scale=1.0 / HD, bias=epsb[:])
                    rstd = nb.tile([128, 512], f32, name=f"rstd_{h}_{qc}",
                                   tag="rstd")
                    nc.vector.reciprocal_approx_fast(rstd[:], sd[:])
                    nc.vector.tensor_mul(usl, usl, rstd[:])

            emit_norm(0)
            for qc in range(NB):
                # normalize for qc+1 queues ahead of Y(qc)'s psum drains so
                # the next column's U is ready before its first matmul
                if qc + 1 < NB:
                    emit_norm(qc + 1)
                for oc in range(NKC):
                    py = yp.tile([128, 512], f32, name=f"py_{oc}_{qc}", tag="py")
                    for h in range(HPC):
                        nc.tensor.matmul(
                            py[:],
                            wo[h][:, oc * 128:(oc + 1) * 128],
                            U[h][:, qc * 512:(qc + 1) * 512],
                            start=(h == 0), stop=(h == HPC - 1))
                    yst = ys.tile([128, 512], bf16, name=f"yst_{oc}_{qc}",
                                  tag="yst")
                    if (oc + qc) % 2 == 0:
                        nc.vector.tensor_copy(yst[:], py[:])
                    else:
                        nc.scalar.copy(yst[:], py[:])
                    nc.sync.dma_start(
                        out=y_t[oc * 128:(oc + 1) * 128, qc * 512:(qc + 1) * 512],
                        in_=yst[:])

    nc.compile()
    return nc


def _host_prep(q, k, v, Wq, Wk, Wv, Wo, lambda_q1, lambda_k1, lambda_q2,
               lambda_k2, gnorm_w, cos_emb, sin_emb):
    import ml_dtypes

    f32 = np.float32
    bf16 = ml_dtypes.bfloat16
    q = np.asarray(q, f32); k = np.asarray(k, f32); v = np.asarray(v, f32)
    Wq = np.asarray(Wq, f32); Wk = np.asarray(Wk, f32)
    Wv = np.asarray(Wv, f32); Wo = np.asarray(Wo, f32)
    gnorm_w = np.asarray(gnorm_w, f32)
    cos_emb = np.asarray(cos_emb, f32); sin_emb = np.asarray(sin_emb, f32)

    lam1 = np.exp(np.sum(np.asarray(lambda_q1, f32) * np.asarray(lambda_k1, f32),
                         dtype=f32))
    lam2 = np.exp(np.sum(np.asarray(lambda_q2, f32) * np.asarray(lambda_k2, f32),
                         dtype=f32))
    lam = np.float32(lam1 - lam2 + LAMBDA_INIT)

    # per-batch transposed activations (bf16)
    xt = {}
    for b in range(B):
        xt[("q", b)] = np.ascontiguousarray(q[b].T).astype(bf16)
        xt[("k", b)] = np.ascontiguousarray(k[b].T).astype(bf16)
        xt[("v", b)] = np.ascontiguousarray(v[b].T).astype(bf16)

    # shared constant tensors
    base_c = cos_emb[:S, :QKD]          # [S, 64]
    base_s = sin_emb[:S, :QKD]
    crep = np.ascontiguousarray(np.tile(base_c.T, (2, 1))).astype(bf16)  # [128, S]
    srep = np.ascontiguousarray(np.tile(base_s.T, (2, 1))).astype(bf16)
    pmat = np.zeros((128, 128), f32)
    for blk in range(2):
        o = blk * 64
        for i in range(QKD // 2):
            pmat[o + 2 * i, o + 2 * i + 1] = 1.0     # lhsT[2i, 2i+1]
            pmat[o + 2 * i + 1, o + 2 * i] = -1.0    # lhsT[2i+1, 2i]
    pmat = pmat.astype(bf16)
    # tri01[p, n] = 0 if p > n (key index > query index within the 128 block)
    tri01 = (np.triu(np.ones((128, 128), f32), 0)).astype(bf16)
    ones_a = np.ones((128, 128), bf16)
    o64 = np.full((128, 128), 1.0 / 64, bf16)
    neglam = np.full((128, 1), -lam, f32)

    per_core = []
    for c in range(NCORES):
        b, grp = c // GRPS, c % GRPS
        heads = [HPC * grp + j for j in range(HPC)]
        # wq/wk columns: tile h; partitions [64g, 64g+64) = map g of head h,
        # original feature order (interleaved pairs)
        cols = []
        for h in range(HPC):
            hg = heads[h]
            for g in range(2):
                cols.extend(hg * HD + g * QKD + d for d in range(QKD))
        cols = np.asarray(cols)
        vrows = np.asarray([h * HD + d for h in heads for d in range(HD)])
        wq_t = np.ascontiguousarray(Wq[cols, :].T).astype(bf16)
        wk_t = np.ascontiguousarray(Wk[cols, :].T).astype(bf16)
        wv_t = np.ascontiguousarray(Wv[vrows, :].T).astype(bf16)
        gtile = np.tile(gnorm_w, HPC)                       # [512]
        wo_t = np.ascontiguousarray(
            ((1.0 - LAMBDA_INIT) * Wo[:, vrows] * gtile[None, :]).T).astype(bf16)
        per_core.append({
            "xq_t": xt[("q", b)], "xk_t": xt[("k", b)], "xv_t": xt[("v", b)],
            "wq_t": wq_t, "wk_t": wk_t, "wv_t": wv_t, "wo_t": wo_t,
            "crep": crep, "srep": srep, "pmat": pmat,
            "tri01": tri01, "ones_a": ones_a, "o64": o64, "neglam": neglam,
        })
    return per_core


def _install_ntff_hook():
    """antenv.axon_hooks is absent in this image; synthesize it so
    run_bass_kernel_spmd(trace=True) can capture NTFF profiles."""
    import sys as _sys
    import types

    if "antenv.axon_hooks" in _sys.modules:
        return
    import antenv
    mod = types.ModuleType("antenv.axon_hooks")
    state = {"hook": None}
    mod.set_axon_ntff_profile_hook = lambda h: state.__setitem__("hook", h)
    mod.get_axon_ntff_profile_hook = lambda: state["hook"]
    _sys.modules["antenv.axon_hooks"] = mod
    antenv.axon_hooks = mod
    try:
        from trn_agent_boot.trn_boot import _ntff_profile_via_ctypes
        state["hook"] = _ntff_profile_via_ctypes("/opt/axon/libaxon_pjrt.so")
    except Exception as e:  # degrade: trace skipped, run still works
        print("ntff hook install failed:", e)


def kernel(q, k, v, Wq, Wk, Wv, Wo, lambda_q1, lambda_k1, lambda_q2,
           lambda_k2, gnorm_w, cos_emb, sin_emb, mask, _trace=False):
    if _trace:
        _install_ntff_hook()
    global _PROGRAM
    if _PROGRAM is None:
        _PROGRAM = _build_program()
    nc = _PROGRAM

    in_maps = _host_prep(q, k, v, Wq, Wk, Wv, Wo, lambda_q1, lambda_k1,
                         lambda_q2, lambda_k2, gnorm_w, cos_emb, sin_emb)

    from concourse.bass_utils import run_bass_kernel_spmd
    res = run_bass_kernel_spmd(nc, in_maps, core_ids=list(range(NCORES)),
                               trace=_trace)
    kernel.last_result = res

    y = np.zeros((B, S, HID), np.float32)
    for c in range(NCORES):
        y[c // GRPS] += np.asarray(res.results[c]["y_t"]).T.astype(np.float32)
    return y


# revision 46
# speedup vs baseline: 1.1735x; 1.0349x over previous
"""Multi-head differential attention on 8 Trainium2 NeuronCores.

Sharding: data-parallel over batch (B=2) x tensor-parallel over heads
(16 heads -> 4 per core). Core c handles batch c//4 and heads
4*(c%4) .. 4*(c%4)+3. Each core computes its heads' attention output and a
partial output projection; the host sums the 4 partials per batch.

v3: bf16 matmul operands, map-packed Q/K (concurrent K=64 score matmuls),
software-pipelined attention emission (PV/SM staggered 2 chunks behind
scores/exp so the in-order PE queue never waits on the scalar engine),
lazy weight DMAs so the first matmul isn't stuck behind the full DMA
trigger queue, batched RMS sqrt (one ACT table switch instead of 32),
rope interleaved into the projection phases.
"""

import math
import os
import sys

sys.path.insert(0, "/opt/trn_rl_repo")

import numpy as np

B, S, HID, NH = 2, 2048, 2048, 16
HD = HID // NH          # 128
QKD = HD // 2           # 64
NCORES = 8
GRPS = NCORES // B      # head groups per batch
HPC = NH // GRPS        # heads per core = 4
LAYER_ID = 1
LAMBDA_INIT = 0.8 - 0.6 * math.exp(-0.3 * LAYER_ID)
EPS = 1e-6

NB = S // 512           # 4 seq blocks of 512
NKC = S // 128          # 16 key chunks of 128

_PROGRAM = None         # compiled bass program, reused across calls


def _build_program():
    import concourse.bass as bass
    import concourse.tile as tile
    from concourse import bacc, mybir

    f32 = mybir.dt.float32
    f32r = mybir.dt.float32r
    bf16 = mybir.dt.bfloat16
    Alu = mybir.AluOpType
    Act = mybir.ActivationFunctionType

    nc = bacc.Bacc(None, target_bir_lowering=False, debug=False)

    def din(name, shape, dt=bf16):
        return nc.dram_tensor(name, shape, dt, kind="ExternalInput").ap()

    io = {
        "xq_t": din("xq_t", [HID, S]),
        "xk_t": din("xk_t", [HID, S]),
        "xv_t": din("xv_t", [HID, S]),
        "wq_t": din("wq_t", [HID, 512]),
        "wk_t": din("wk_t", [HID, 512]),
        "wv_t": din("wv_t", [HID, 512]),
        "wo_t": din("wo_t", [512, HID]),
        "crep": din("crep", [128, S]),
        "srep": din("srep", [128, S]),
        "pmat": din("pmat", [128, 128]),
        "tri01": din("tri01", [128, 128]),
        "ones_a": din("ones_a", [128, 128]),
        "o64": din("o64", [128, 128]),
        "neglam": din("neglam", [128, 1], f32),
    }
    y_t = nc.dram_tensor("y_t", [HID, S], bf16, kind="ExternalOutput").ap()

    from contextlib import ExitStack

    with tile.TileContext(nc) as tc, ExitStack() as ctx:
        persist = ctx.enter_context(tc.tile_pool(name="persist", bufs=1))
        constp = ctx.enter_context(tc.tile_pool(name="constp", bufs=1))

        # constants
        crep = constp.tile([128, S], bf16, name="crep_sb", tag="crep")
        srep = constp.tile([128, S], bf16, name="srep_sb", tag="srep")
        pmat = constp.tile([128, 128], bf16, name="pmat_sb", tag="pmat")
        tri01 = constp.tile([128, 128], bf16, name="tri01_sb", tag="tri01")
        ones_a = constp.tile([128, 128], bf16, name="ones_a_sb", tag="ones_a")
        o64 = constp.tile([128, 128], bf16, name="o64_sb", tag="o64")
        neglam = constp.tile([128, 1], f32, name="neglam_sb", tag="neglam")
        epsb = constp.tile([128, 1], f32, name="epsb", tag="epsb")
        nc.vector.memset(epsb[:], EPS)

        def emit_const_dmas():
            # deferred onto the scalar engine's DGE queue (behind the wq
            # chunks): keeps 1MB of table transfers out of the sync queue so
            # neither the first matmul nor the x-chunk stream waits on them
            for t, key in ((crep, "crep"), (srep, "srep"), (pmat, "pmat"),
                           (tri01, "tri01"), (ones_a, "ones_a"),
                           (o64, "o64"), (neglam, "neglam")):
                nc.scalar.dma_start(out=t[:], in_=io[key][:])

        # persistent tensors, per head h: Q^T/K^T [128, S] bf16 where
        # partitions [64g, 64g+64) hold softmax-map g's features.
        QT = [persist.tile([128, S], bf16, name=f"qt{h}", tag=f"qt{h}")
              for h in range(HPC)]
        KT = [persist.tile([128, S], bf16, name=f"kt{h}", tag=f"kt{h}")
              for h in range(HPC)]
        # V natural layout per 128-seq chunk: [128 seq, 4 heads * 128 feat]
        VH = [persist.tile([128, 512], bf16, name=f"vh{s}", tag=f"vh{s}")
              for s in range(NKC)]
        # combined attention output, transposed: [feat, seq]
        U = [persist.tile([128, S], bf16, name=f"u{h}", tag=f"u{h}")
             for h in range(HPC)]
        # output projection weights + per-(h,qb) sum-of-squares staging
        wo = [persist.tile([128, S], bf16, name=f"wo{h}", tag=f"wo{h}")
              for h in range(HPC)]
        ssqb = [persist.tile([128, 512], bf16, name=f"ssqb{i}", tag=f"ssqb{i}")
                for i in range(HPC * NB)]

        # ------------- phase P + R: projections with rope interleaved -------
        # rope: y = x*cos + (P x)*sin, P = signed pair-swap (PE matmul).
        rope_q = []   # pending (tiles, n) rope tasks

        with tc.tile_pool(name="wp", bufs=1) as wp, \
             tc.tile_pool(name="xp", bufs=6) as xp, \
             tc.tile_pool(name="rtp", bufs=4) as rtp, \
             tc.tile_pool(name="pp", bufs=1, space="PSUM") as pp:

            def emit_rope():
                T4, n = rope_q.pop(0)
                sl = slice(n * 512, (n + 1) * 512)
                pxs = []
                for h in range(HPC):
                    px = pp.tile([128, 512], f32, name=f"px_{T4[h].name}_{n}",
                                 tag="px", bufs=3)
                    nc.tensor.matmul(px[:], pmat[:], T4[h][:, sl],
                                     start=True, stop=True)
                    pxs.append(px)
                for h in range(HPC):
                    tmp = rtp.tile([128, 512], bf16,
                                   name=f"rs_{T4[h].name}_{n}", tag="rs")
                    nc.vector.tensor_mul(tmp[:], pxs[h][:], srep[:, sl])
                    tmp2 = rtp.tile([128, 512], bf16,
                                    name=f"rc_{T4[h].name}_{n}", tag="rc")
                    nc.vector.tensor_mul(tmp2[:], T4[h][:, sl], crep[:, sl])
                    nc.vector.tensor_add(T4[h][:, sl], tmp[:], tmp2[:])

            wnames = ("wq_t", "wk_t", "wv_t")
            wts = {m: [wp.tile([128, 512], bf16, name=f"{wnames[m]}_{kc}",
                               tag=f"w{m}_{kc}") for kc in range(NKC)]
                   for m in range(3)}

            def wdma(m, kc):
                # weight DMAs ride the scalar engine's DGE queue so they
                # never contend with the x-chunk triggers on sync
                nc.scalar.dma_start(out=wts[m][kc][:],
                                    in_=io[wnames[m]][kc * 128:(kc + 1) * 128, :])

            for mode, xname in enumerate(("xq_t", "xk_t", "xv_t")):
                wt = wts[mode]
                xin = io[xname]
                for n in range(NB):
                    blk_idx = mode * NB + n
                    ps = [pp.tile([128, 512], f32, name=f"pp{t}_{mode}_{n}",
                                  tag="pp", bufs=5) for t in range(4)]
                    for kc in range(NKC):
                        if mode == 0 and n == 0:
                            wdma(0, kc)       # lazy: wq arrives just ahead
                        xck = xp.tile([128, 512], bf16, name=f"x_{mode}_{n}_{kc}",
                                      tag="x")
                        nc.sync.dma_start(
                            out=xck[:],
                            in_=xin[kc * 128:(kc + 1) * 128,
                                    n * 512:(n + 1) * 512])
                        if mode == 0 and n == 1:
                            wdma(1, kc)       # wk during q-proj
                        elif mode == 0 and n == 2:
                            wdma(2, kc)       # wv during q-proj
                        elif mode == 0 and n == 3 and kc % 4 == 0:
                            h = kc // 4       # wo prefetch, needed in phase Y
                            nc.scalar.dma_start(
                                out=wo[h][:],
                                in_=io["wo_t"][h * 128:(h + 1) * 128, :])
                        for t in range(4):
                            if mode < 2:
                                lhsT = wt[kc][:, t * 128:(t + 1) * 128]
                                rhs = xck[:]
                            else:
                                lhsT = xck[:, t * 128:(t + 1) * 128]
                                rhs = wt[kc][:]
                            nc.tensor.matmul(ps[t][:], lhsT, rhs,
                                             start=(kc == 0), stop=(kc == NKC - 1))
                    # drains on the scalar engine (vector is busy with rope)
                    for t in range(4):
                        if mode == 0:
                            nc.scalar.copy(QT[t][:, n * 512:(n + 1) * 512],
                                           ps[t][:])
                        elif mode == 1:
                            nc.scalar.copy(KT[t][:, n * 512:(n + 1) * 512],
                                           ps[t][:])
                        else:
                            nc.scalar.copy(VH[n * 4 + t][:], ps[t][:])
                    if blk_idx == 0:
                        emit_const_dmas()
                    if mode == 0:
                        rope_q.append((QT, n))
                    elif mode == 1:
                        rope_q.append((KT, n))
                    if blk_idx >= 2 and rope_q and blk_idx - (
                            rope_q[0][1] + (0 if rope_q[0][0] is QT else NB)) >= 2:
                        emit_rope()
            while rope_q:
                emit_rope()

        # ------------- phase A: attention, PV/SM staggered 2 chunks ---------
        with tc.tile_pool(name="spp", bufs=1, space="PSUM") as spp, \
             tc.tile_pool(name="pvp", bufs=1, space="PSUM") as pvp, \
             tc.tile_pool(name="smp", bufs=1, space="PSUM") as smp, \
             tc.tile_pool(name="ep", bufs=5) as ep, \
             tc.tile_pool(name="cb", bufs=2) as cb:
            rms_pending = None

            for h in range(HPC):
                for qb in range(NB):
                    pv = pvp.tile([128, 1024], f32, name=f"pv_{h}_{qb}",
                                  tag="pv")
                    sm = smp.tile([128, 1024], f32, name=f"sm_{h}_{qb}",
                                  tag="sm")
                    nkc = 4 * qb + 4
                    pvsm_pending = []

                    def make_pvsm(kc, E, qoff, first, last, h=h, pv=pv, sm=sm):
                        def emit():
                            for g in (0, 1):
                                nc.tensor.matmul(
                                    pv[:, g * 512 + qoff:g * 512 + 512],
                                    VH[kc][:, h * 128:(h + 1) * 128],
                                    E[:, g * 512 + qoff:g * 512 + 512],
                                    start=first, stop=last)
                            for g in (0, 1):
                                nc.tensor.matmul(
                                    sm[:, g * 512 + qoff:g * 512 + 512],
                                    ones_a[:],
                                    E[:, g * 512 + qoff:g * 512 + 512],
                                    start=first, stop=last)
                        return emit

                    for kc in range(nkc):
                        j = kc - 4 * qb  # >= 0 on the causal diagonal band
                        qoff = j * 128 if j >= 0 else 0
                        sp = spp.tile([128, 1024], f32, name=f"s_{h}_{qb}_{kc}",
                                      tag="sp", bufs=2)
                        for g in (0, 1):
                            goff = g * 512
                            nc.tensor.matmul(
                                sp[:, goff + qoff:goff + 512],
                                KT[h][g * 64:(g + 1) * 64,
                                      kc * 128:(kc + 1) * 128],
                                QT[h][g * 64:(g + 1) * 64,
                                      qb * 512 + qoff:(qb + 1) * 512],
                                start=True, stop=True)
                        E = ep.tile([128, 1024], bf16, name=f"e_{h}_{qb}_{kc}",
                                    tag="e")
                        if qoff == 0:
                            nc.scalar.activation(E[:], sp[:], Act.Exp,
                                                 scale=0.125)
                        else:
                            for g in (0, 1):
                                nc.scalar.activation(
                                    E[:, g * 512 + qoff:g * 512 + 512],
                                    sp[:, g * 512 + qoff:g * 512 + 512],
                                    Act.Exp, scale=0.125)
                        if j >= 0:
                            # causal mask inside the diagonal 128x128 block,
                            # applied multiplicatively on the vector engine
                            for g in (0, 1):
                                esl = E[:, g * 512 + qoff:g * 512 + qoff + 128]
                                nc.vector.tensor_mul(esl, esl, tri01[:])
                        pvsm_pending.append(make_pvsm(
                            kc, E, qoff, kc == 0, kc == nkc - 1))
                        if len(pvsm_pending) > 3:
                            pvsm_pending.pop(0)()
                        if kc == 1 and rms_pending is not None:
                            rms_pending()
                            rms_pending = None
                    while pvsm_pending:
                        pvsm_pending.pop(0)()

                    # combine: U = pv0/sm0 - lam * pv1/sm1
                    usl = U[h][:, qb * 512:(qb + 1) * 512]
                    rb = cb.tile([128, 1024], f32, name=f"rb_{h}_{qb}", tag="rb")
                    nc.vector.reciprocal_approx_fast(rb[:], sm[:])
                    t1 = cb.tile([128, 512], f32, name=f"t1_{h}_{qb}", tag="t1")
                    t2 = cb.tile([128, 512], f32, name=f"t2_{h}_{qb}", tag="t2")
                    nc.vector.tensor_mul(t1[:], pv[:, 0:512], rb[:, 0:512])
                    nc.vector.tensor_mul(t2[:], pv[:, 512:1024], rb[:, 512:1024])
                    nc.vector.scalar_tensor_tensor(
                        usl, t2[:], neglam[:], t1[:],
                        op0=Alu.mult, op1=Alu.add)
                    # reserve the sm-tag psum bank for this iteration's ssq
                    # before the next iteration's sm allocation
                    ssq = smp.tile([128, 512], f32, name=f"ssq_{h}_{qb}",
                                   tag="sm")

                    def make_rms(h=h, qb=qb, usl=usl, ssq=ssq):
                        def emit():
                            sq = cb.tile([128, 512], bf16, name=f"sq_{h}_{qb}",
                                         tag="sq")
                            nc.vector.tensor_mul(sq[:], usl, usl)
                            nc.tensor.matmul(ssq[:], ones_a[:], sq[:],
                                             start=True, stop=True)
                            nc.vector.tensor_copy(ssqb[h * NB + qb][:], ssq[:])
                        return emit
                    rms_pending = make_rms()
            if rms_pending is not None:
                rms_pending()
                rms_pending = None

        # ------------- phase N + Y: batched RMS norm, output projection -----
        with tc.tile_pool(name="yp", bufs=4, space="PSUM") as yp, \
             tc.tile_pool(name="ys", bufs=4) as ys, \
             tc.tile_pool(name="nb", bufs=4) as nb:
            def emit_norm(qc):
                for h in range(HPC):
                    usl = U[h][:, qc * 512:(qc + 1) * 512]
                    sd = nb.tile([128, 512], f32, name=f"sd_{h}_{qc}", tag="sd")
                    nc.scalar.activation(sd[:], ssqb[h * NB + qc][:], Act.Sqrt,
                                         scale=1.0 / HD, bias=epsb[:])
                    rstd = nb.tile([128, 512], f32, name=f"rstd_{h}_{qc}",
                                   tag="rstd")
                    nc.vector.reciprocal_approx_fast(rstd[:], sd[:])
                    nc.vector.tensor_mul(usl, usl, rstd[:])

            emit_norm(0)
            for qc in range(NB):
                # normalize for qc+1 queues ahead of Y(qc)'s psum drains so
                # the next column's U is ready before its first matmul
                if qc + 1 < NB:
                    emit_norm(qc + 1)
                for oc in range(NKC):
                    py = yp.tile([128, 512], f32, name=f"py_{oc}_{qc}", tag="py")
                    for h in range(HPC):
                        nc.tensor.matmul(
                            py[:],
                            wo[h][:, oc * 128:(oc + 1) * 128],
                            U[h][:, qc * 512:(qc + 1) * 512],
                            start=(h == 0), stop=(h == HPC - 1))
                    yst = ys.tile([128, 512], bf16, name=f"yst_{oc}_{qc}",
                                  tag="yst")
                    if (oc + qc) % 2 == 0:
                        nc.vector.tensor_copy(yst[:], py[:])
                    else:
                        nc.scalar.copy(yst[:], py[:])
                    nc.sync.dma_start(
                        out=y_t[oc * 128:(oc + 1) * 128, qc * 512:(qc + 1) * 512],
                        in_=yst[:])

    nc.compile()
    return nc


def _host_prep(q, k, v, Wq, Wk, Wv, Wo, lambda_q1, lambda_k1, lambda_q2,
               lambda_k2, gnorm_w, cos_emb, sin_emb):
    import ml_dtypes

    f32 = np.float32
    bf16 = ml_dtypes.bfloat16
    q = np.asarray(q, f32); k = np.asarray(k, f32); v = np.asarray(v, f32)
    Wq = np.asarray(Wq, f32); Wk = np.asarray(Wk, f32)
    Wv = np.asarray(Wv, f32); Wo = np.asarray(Wo, f32)
    gnorm_w = np.asarray(gnorm_w, f32)
    cos_emb = np.asarray(cos_emb, f32); sin_emb = np.asarray(sin_emb, f32)

    lam1 = np.exp(np.sum(np.asarray(lambda_q1, f32) * np.asarray(lambda_k1, f32),
                         dtype=f32))
    lam2 = np.exp(np.sum(np.asarray(lambda_q2, f32) * np.asarray(lambda_k2, f32),
                         dtype=f32))
    lam = np.float32(lam1 - lam2 + LAMBDA_INIT)

    # per-batch transposed activations (bf16)
    xt = {}
    for b in range(B):
        xt[("q", b)] = np.ascontiguousarray(q[b].T).astype(bf16)
        xt[("k", b)] = np.ascontiguousarray(k[b].T).astype(bf16)
        xt[("v", b)] = np.ascontiguousarray(v[b].T).astype(bf16)

    # shared constant tensors
    base_c = cos_emb[:S, :QKD]          # [S, 64]
    base_s = sin_emb[:S, :QKD]
    crep = np.ascontiguousarray(np.tile(base_c.T, (2, 1))).astype(bf16)  # [128, S]
    srep = np.ascontiguousarray(np.tile(base_s.T, (2, 1))).astype(bf16)
    pmat = np.zeros((128, 128), f32)
    for blk in range(2):
        o = blk * 64
        for i in range(QKD // 2):
            pmat[o + 2 * i, o + 2 * i + 1] = 1.0     # lhsT[2i, 2i+1]
            pmat[o + 2 * i + 1, o + 2 * i] = -1.0    # lhsT[2i+1, 2i]
    pmat = pmat.astype(bf16)
    # tri01[p, n] = 0 if p > n (key index > query index within the 128 block)
    tri01 = (np.triu(np.ones((128, 128), f32), 0)).astype(bf16)
    ones_a = np.ones((128, 128), bf16)
    o64 = np.full((128, 128), 1.0 / 64, bf16)
    neglam = np.full((128, 1), -lam, f32)

    per_core = []
    for c in range(NCORES):
        b, grp = c // GRPS, c % GRPS
        heads = [HPC * grp + j for j in range(HPC)]
        # wq/wk columns: tile h; partitions [64g, 64g+64) = map g of head h,
        # original feature order (interleaved pairs)
        cols = []
        for h in range(HPC):
            hg = heads[h]
            for g in range(2):
                cols.extend(hg * HD + g * QKD + d for d in range(QKD))
        cols = np.asarray(cols)
        vrows = np.asarray([h * HD + d for h in heads for d in range(HD)])
        wq_t = np.ascontiguousarray(Wq[cols, :].T).astype(bf16)
        wk_t = np.ascontiguousarray(Wk[cols, :].T).astype(bf16)
        wv_t = np.ascontiguousarray(Wv[vrows, :].T).astype(bf16)
        gtile = np.tile(gnorm_w, HPC)                       # [512]
        wo_t = np.ascontiguousarray(
            ((1.0 - LAMBDA_INIT) * Wo[:, vrows] * gtile[None, :]).T).astype(bf16)
        per_core.append({
            "xq_t": xt[("q", b)], "xk_t": xt[("k", b)], "xv_t": xt[("v", b)],
            "wq_t": wq_t, "wk_t": wk_t, "wv_t": wv_t, "wo_t": wo_t,
            "crep": crep, "srep": srep, "pmat": pmat,
            "tri01": tri01, "ones_a": ones_a, "o64": o64, "neglam": neglam,
        })
    return per_core


def _install_ntff_hook():
    """antenv.axon_hooks is absent in this image; synthesize it so
    run_bass_kernel_spmd(trace=True) can capture NTFF profiles."""
    import sys as _sys
    import types

    if "antenv.axon_hooks" in _sys.modules:
        return
    import antenv
    mod = types.ModuleType("antenv.axon_hooks")
    state = {"hook": None}
    mod.set_axon_ntff_profile_hook = lambda h: state.__setitem__("hook", h)
    mod.get_axon_ntff_profile_hook = lambda: state["hook"]
    _sys.modules["antenv.axon_hooks"] = mod
    antenv.axon_hooks = mod
    try:
        from trn_agent_boot.trn_boot import _ntff_profile_via_ctypes
        state["hook"] = _ntff_profile_via_ctypes("/opt/axon/libaxon_pjrt.so")
    except Exception as e:  # degrade: trace skipped, run still works
        print("ntff hook install failed:", e)


def kernel(q, k, v, Wq, Wk, Wv, Wo, lambda_q1, lambda_k1, lambda_q2,
           lambda_k2, gnorm_w, cos_emb, sin_emb, mask, _trace=False):
    if _trace:
        _install_ntff_hook()
    global _PROGRAM
    if _PROGRAM is None:
        _PROGRAM = _build_program()
    nc = _PROGRAM

    in_maps = _host_prep(q, k, v, Wq, Wk, Wv, Wo, lambda_q1, lambda_k1,
                         lambda_q2, lambda_k2, gnorm_w, cos_emb, sin_emb)

    from concourse.bass_utils import run_bass_kernel_spmd
    res = run_bass_kernel_spmd(nc, in_maps, core_ids=list(range(NCORES)),
                               trace=_trace)
    kernel.last_result = res

    y = np.zeros((B, S, HID), np.float32)
    for c in range(NCORES):
        y[c // GRPS] += np.asarray(res.results[c]["y_t"]).T.astype(np.float32)
    return y


# revision 53
# speedup vs baseline: 1.2060x; 1.0276x over previous
"""Multi-head differential attention on 8 Trainium2 NeuronCores.

Sharding: data-parallel over batch (B=2) x tensor-parallel over heads
(16 heads -> 4 per core). Core c handles batch c//4 and heads
4*(c%4) .. 4*(c%4)+3. Each core computes its heads' attention output and a
partial output projection; the host sums the 4 partials per batch.

v3: bf16 matmul operands, map-packed Q/K (concurrent K=64 score matmuls),
software-pipelined attention emission (PV/SM staggered 2 chunks behind
scores/exp so the in-order PE queue never waits on the scalar engine),
lazy weight DMAs so the first matmul isn't stuck behind the full DMA
trigger queue, batched RMS sqrt (one ACT table switch instead of 32),
rope interleaved into the projection phases.
"""

import math
import os
import sys

sys.path.insert(0, "/opt/trn_rl_repo")

import numpy as np

B, S, HID, NH = 2, 2048, 2048, 16
HD = HID // NH          # 128
QKD = HD // 2           # 64
NCORES = 8
GRPS = NCORES // B      # head groups per batch
HPC = NH // GRPS        # heads per core = 4
LAYER_ID = 1
LAMBDA_INIT = 0.8 - 0.6 * math.exp(-0.3 * LAYER_ID)
EPS = 1e-6

NB = S // 512           # 4 seq blocks of 512
NKC = S // 128          # 16 key chunks of 128

_PROGRAM = None         # compiled bass program, reused across calls


def _build_program():
    import concourse.bass as bass
    import concourse.tile as tile
    from concourse import bacc, mybir

    f32 = mybir.dt.float32
    f32r = mybir.dt.float32r
    bf16 = mybir.dt.bfloat16
    Alu = mybir.AluOpType
    Act = mybir.ActivationFunctionType

    nc = bacc.Bacc(None, target_bir_lowering=False, debug=False)

    def din(name, shape, dt=bf16):
        return nc.dram_tensor(name, shape, dt, kind="ExternalInput").ap()

    io = {
        "xq_t": din("xq_t", [HID, S]),
        "xk_t": din("xk_t", [HID, S]),
        "xv_t": din("xv_t", [HID, S]),
        "wq_t": din("wq_t", [HID, 512]),
        "wk_t": din("wk_t", [HID, 512]),
        "wv_t": din("wv_t", [HID, 512]),
        "wo_t": din("wo_t", [512, HID]),
        "crep": din("crep", [128, S]),
        "srep": din("srep", [128, S]),
        "pmat": din("pmat", [128, 128]),
        "tri01": din("tri01", [128, 128]),
        "ones_a": din("ones_a", [128, 128]),
        "o64": din("o64", [128, 128]),
        "neglam": din("neglam", [128, 1], f32),
    }
    y_t = nc.dram_tensor("y_t", [HID, S], bf16, kind="ExternalOutput").ap()

    from contextlib import ExitStack

    with tile.TileContext(nc) as tc, ExitStack() as ctx:
        persist = ctx.enter_context(tc.tile_pool(name="persist", bufs=1))
        constp = ctx.enter_context(tc.tile_pool(name="constp", bufs=1))

        # constants
        crep = constp.tile([128, S], bf16, name="crep_sb", tag="crep")
        srep = constp.tile([128, S], bf16, name="srep_sb", tag="srep")
        pmat = constp.tile([128, 128], bf16, name="pmat_sb", tag="pmat")
        tri01 = constp.tile([128, 128], bf16, name="tri01_sb", tag="tri01")
        ones_a = constp.tile([128, 128], bf16, name="ones_a_sb", tag="ones_a")
        o64 = constp.tile([128, 128], bf16, name="o64_sb", tag="o64")
        neglam = constp.tile([128, 1], f32, name="neglam_sb", tag="neglam")
        epsb = constp.tile([128, 1], f32, name="epsb", tag="epsb")
        nc.vector.memset(epsb[:], EPS)

        def emit_const_dmas():
            # deferred onto the scalar engine's DGE queue (behind the wq
            # chunks): keeps 1MB of table transfers out of the sync queue so
            # neither the first matmul nor the x-chunk stream waits on them
            for t, key in ((crep, "crep"), (srep, "srep"), (pmat, "pmat"),
                           (tri01, "tri01"), (ones_a, "ones_a"),
                           (o64, "o64"), (neglam, "neglam")):
                nc.scalar.dma_start(out=t[:], in_=io[key][:])

        # persistent tensors, per head h: Q^T/K^T [128, S] bf16 where
        # partitions [64g, 64g+64) hold softmax-map g's features.
        QT = [persist.tile([128, S], bf16, name=f"qt{h}", tag=f"qt{h}")
              for h in range(HPC)]
        KT = [persist.tile([128, S], bf16, name=f"kt{h}", tag=f"kt{h}")
              for h in range(HPC)]
        # V natural layout per 128-seq chunk: [128 seq, 4 heads * 128 feat]
        VH = [persist.tile([128, 512], bf16, name=f"vh{s}", tag=f"vh{s}")
              for s in range(NKC)]
        # combined attention output, transposed: [feat, seq]
        U = [persist.tile([128, S], bf16, name=f"u{h}", tag=f"u{h}")
             for h in range(HPC)]
        # output projection weights (prefetched early)
        wo = [persist.tile([128, S], bf16, name=f"wo{h}", tag=f"wo{h}")
              for h in range(HPC)]

        # ------------- phase P + R: projections with rope interleaved -------
        # rope: y = x*cos + (P x)*sin, P = signed pair-swap (PE matmul).
        rope_q = []   # pending (tiles, n) rope tasks

        with tc.tile_pool(name="wp", bufs=1) as wp, \
             tc.tile_pool(name="xp", bufs=6) as xp, \
             tc.tile_pool(name="rtp", bufs=4) as rtp, \
             tc.tile_pool(name="pp", bufs=1, space="PSUM") as pp:

            def emit_rope():
                T4, n = rope_q.pop(0)
                sl = slice(n * 512, (n + 1) * 512)
                pxs = []
                for h in range(HPC):
                    px = pp.tile([128, 512], f32, name=f"px_{T4[h].name}_{n}",
                                 tag="px", bufs=3)
                    nc.tensor.matmul(px[:], pmat[:], T4[h][:, sl],
                                     start=True, stop=True)
                    pxs.append(px)
                for h in range(HPC):
                    tmp = rtp.tile([128, 512], bf16,
                                   name=f"rs_{T4[h].name}_{n}", tag="rs")
                    nc.vector.tensor_mul(tmp[:], pxs[h][:], srep[:, sl])
                    tmp2 = rtp.tile([128, 512], bf16,
                                    name=f"rc_{T4[h].name}_{n}", tag="rc")
                    nc.vector.tensor_mul(tmp2[:], T4[h][:, sl], crep[:, sl])
                    nc.vector.tensor_add(T4[h][:, sl], tmp[:], tmp2[:])

            wnames = ("wq_t", "wk_t", "wv_t")
            wts = {m: [wp.tile([128, 512], bf16, name=f"{wnames[m]}_{kc}",
                               tag=f"w{m}_{kc}") for kc in range(NKC)]
                   for m in range(3)}

            def wdma(m, kc):
                # weight DMAs ride the scalar engine's DGE queue so they
                # never contend with the x-chunk triggers on sync
                nc.scalar.dma_start(out=wts[m][kc][:],
                                    in_=io[wnames[m]][kc * 128:(kc + 1) * 128, :])

            for mode, xname in enumerate(("xq_t", "xk_t", "xv_t")):
                wt = wts[mode]
                xin = io[xname]
                for n in range(NB):
                    blk_idx = mode * NB + n
                    ps = [pp.tile([128, 512], f32, name=f"pp{t}_{mode}_{n}",
                                  tag="pp", bufs=5) for t in range(4)]
                    for kc in range(NKC):
                        if mode == 0 and n == 0:
                            wdma(0, kc)       # lazy: wq arrives just ahead
                        xck = xp.tile([128, 512], bf16, name=f"x_{mode}_{n}_{kc}",
                                      tag="x")
                        nc.sync.dma_start(
                            out=xck[:],
                            in_=xin[kc * 128:(kc + 1) * 128,
                                    n * 512:(n + 1) * 512])
                        if mode == 0 and n == 1:
                            wdma(1, kc)       # wk during q-proj
                        elif mode == 0 and n == 2:
                            wdma(2, kc)       # wv during q-proj
                        elif mode == 0 and n == 3 and kc % 4 == 0:
                            h = kc // 4       # wo prefetch, needed in phase Y
                            nc.scalar.dma_start(
                                out=wo[h][:],
                                in_=io["wo_t"][h * 128:(h + 1) * 128, :])
                        for t in range(4):
                            if mode < 2:
                                lhsT = wt[kc][:, t * 128:(t + 1) * 128]
                                rhs = xck[:]
                            else:
                                lhsT = xck[:, t * 128:(t + 1) * 128]
                                rhs = wt[kc][:]
                            nc.tensor.matmul(ps[t][:], lhsT, rhs,
                                             start=(kc == 0), stop=(kc == NKC - 1))
                    # drains alternate scalar/vector so the next block's psum
                    # reuse isn't gated on a serial scalar-engine drain queue
                    for t in range(4):
                        if mode == 0:
                            dst = QT[t][:, n * 512:(n + 1) * 512]
                        elif mode == 1:
                            dst = KT[t][:, n * 512:(n + 1) * 512]
                        else:
                            dst = VH[n * 4 + t][:]
                        if t % 2 == 0:
                            nc.vector.tensor_copy(dst, ps[t][:])
                        else:
                            nc.scalar.copy(dst, ps[t][:])
                    if blk_idx == 0:
                        emit_const_dmas()
                    if mode == 0:
                        rope_q.append((QT, n))
                    elif mode == 1:
                        rope_q.append((KT, n))
                    if blk_idx >= 2 and rope_q and blk_idx - (
                            rope_q[0][1] + (0 if rope_q[0][0] is QT else NB)) >= 2:
                        emit_rope()
            while rope_q:
                emit_rope()

        # ------------- phase A: attention, PV/SM staggered 2 chunks ---------
        with tc.tile_pool(name="spp", bufs=1, space="PSUM") as spp, \
             tc.tile_pool(name="pvp", bufs=1, space="PSUM") as pvp, \
             tc.tile_pool(name="smp", bufs=1, space="PSUM") as smp, \
             tc.tile_pool(name="ep", bufs=5) as ep, \
             tc.tile_pool(name="cb", bufs=2) as cb:
            for h in range(HPC):
                for qb in range(NB):
                    pv = pvp.tile([128, 1024], f32, name=f"pv_{h}_{qb}",
                                  tag="pv")
                    sm = smp.tile([128, 1024], f32, name=f"sm_{h}_{qb}",
                                  tag="sm")
                    nkc = 4 * qb + 4
                    pvsm_pending = []

                    def make_pvsm(kc, E, qoff, first, last, h=h, pv=pv, sm=sm):
                        def emit():
                            for g in (0, 1):
                                nc.tensor.matmul(
                                    pv[:, g * 512 + qoff:g * 512 + 512],
                                    VH[kc][:, h * 128:(h + 1) * 128],
                                    E[:, g * 512 + qoff:g * 512 + 512],
                                    start=first, stop=last)
                            for g in (0, 1):
                                nc.tensor.matmul(
                                    sm[:, g * 512 + qoff:g * 512 + 512],
                                    ones_a[:],
                                    E[:, g * 512 + qoff:g * 512 + 512],
                                    start=first, stop=last)
                        return emit

                    for kc in range(nkc):
                        j = kc - 4 * qb  # >= 0 on the causal diagonal band
                        qoff = j * 128 if j >= 0 else 0
                        sp = spp.tile([128, 1024], f32, name=f"s_{h}_{qb}_{kc}",
                                      tag="sp", bufs=2)
                        for g in (0, 1):
                            goff = g * 512
                            nc.tensor.matmul(
                                sp[:, goff + qoff:goff + 512],
                                KT[h][g * 64:(g + 1) * 64,
                                      kc * 128:(kc + 1) * 128],
                                QT[h][g * 64:(g + 1) * 64,
                                      qb * 512 + qoff:(qb + 1) * 512],
                                start=True, stop=True)
                        E = ep.tile([128, 1024], bf16, name=f"e_{h}_{qb}_{kc}",
                                    tag="e")
                        if qoff == 0:
                            nc.scalar.activation(E[:], sp[:], Act.Exp,
                                                 scale=0.125)
                        else:
                            for g in (0, 1):
                                nc.scalar.activation(
                                    E[:, g * 512 + qoff:g * 512 + 512],
                                    sp[:, g * 512 + qoff:g * 512 + 512],
                                    Act.Exp, scale=0.125)
                        if j >= 0:
                            # causal mask inside the diagonal 128x128 block,
                            # applied multiplicatively on the vector engine
                            for g in (0, 1):
                                esl = E[:, g * 512 + qoff:g * 512 + qoff + 128]
                                nc.vector.tensor_mul(esl, esl, tri01[:])
                        pvsm_pending.append(make_pvsm(
                            kc, E, qoff, kc == 0, kc == nkc - 1))
                        if len(pvsm_pending) > 3:
                            pvsm_pending.pop(0)()
                    while pvsm_pending:
                        pvsm_pending.pop(0)()

                    # combine: U = pv0/sm0 - lam * pv1/sm1
                    usl = U[h][:, qb * 512:(qb + 1) * 512]
                    rb = cb.tile([128, 1024], f32, name=f"rb_{h}_{qb}", tag="rb")
                    nc.vector.reciprocal_approx_fast(rb[:], sm[:])
                    t1 = cb.tile([128, 512], f32, name=f"t1_{h}_{qb}", tag="t1")
                    t2 = cb.tile([128, 512], f32, name=f"t2_{h}_{qb}", tag="t2")
                    nc.vector.tensor_mul(t1[:], pv[:, 0:512], rb[:, 0:512])
                    nc.vector.tensor_mul(t2[:], pv[:, 512:1024], rb[:, 512:1024])
                    nc.vector.scalar_tensor_tensor(
                        usl, t2[:], neglam[:], t1[:],
                        op0=Alu.mult, op1=Alu.add)

        # ------------- phase N + Y: batched RMS norm, output projection -----
        with tc.tile_pool(name="yp", bufs=4, space="PSUM") as yp, \
             tc.tile_pool(name="sqp", bufs=4, space="PSUM") as sqp, \
             tc.tile_pool(name="ys", bufs=4) as ys, \
             tc.tile_pool(name="nb", bufs=4) as nb:
            def emit_norm(qc):
                for h in range(HPC):
                    usl = U[h][:, qc * 512:(qc + 1) * 512]
                    sq = nb.tile([128, 512], bf16, name=f"sq_{h}_{qc}", tag="sq")
                    nc.vector.tensor_mul(sq[:], usl, usl)
                    ssq = sqp.tile([128, 512], f32, name=f"ssq_{h}_{qc}",
                                   tag="ssq")
                    nc.tensor.matmul(ssq[:], ones_a[:], sq[:],
                                     start=True, stop=True)
                    sd = nb.tile([128, 512], f32, name=f"sd_{h}_{qc}", tag="sd")
                    nc.scalar.activation(sd[:], ssq[:], Act.Sqrt,
                                         scale=1.0 / HD, bias=epsb[:])
                    rstd = nb.tile([128, 512], f32, name=f"rstd_{h}_{qc}",
                                   tag="rstd")
                    nc.vector.reciprocal_approx_fast(rstd[:], sd[:])
                    nc.vector.tensor_mul(usl, usl, rstd[:])

            emit_norm(0)
            for qc in range(NB):
                # normalize for qc+1 queues ahead of Y(qc)'s psum drains so
                # the next column's U is ready before its first matmul
                if qc + 1 < NB:
                    emit_norm(qc + 1)
                for oc in range(NKC):
                    py = yp.tile([128, 512], f32, name=f"py_{oc}_{qc}", tag="py")
                    for h in range(HPC):
                        nc.tensor.matmul(
                            py[:],
                            wo[h][:, oc * 128:(oc + 1) * 128],
                            U[h][:, qc * 512:(qc + 1) * 512],
                            start=(h == 0), stop=(h == HPC - 1))
                    yst = ys.tile([128, 512], bf16, name=f"yst_{oc}_{qc}",
                                  tag="yst")
                    if (oc + qc) % 2 == 0:
                        nc.vector.tensor_copy(yst[:], py[:])
                        eng = nc.sync
                    else:
                        nc.scalar.copy(yst[:], py[:])
                        eng = nc.scalar
                    # alternate DGE queues so the output triggers never back
                    # up behind one engine's serial trigger stream
                    eng.dma_start(
                        out=y_t[oc * 128:(oc + 1) * 128, qc * 512:(qc + 1) * 512],
                        in_=yst[:])

    nc.compile()
    return nc


def _host_prep(q, k, v, Wq, Wk, Wv, Wo, lambda_q1, lambda_k1, lambda_q2,
               lambda_k2, gnorm_w, cos_emb, sin_emb):
    import ml_dtypes

    f32 = np.float32
    bf16 = ml_dtypes.bfloat16
    q = np.asarray(q, f32); k = np.asarray(k, f32); v = np.asarray(v, f32)
    Wq = np.asarray(Wq, f32); Wk = np.asarray(Wk, f32)
    Wv = np.asarray(Wv, f32); Wo = np.asarray(Wo, f32)
    gnorm_w = np.asarray(gnorm_w, f32)
    cos_emb = np.asarray(cos_emb, f32); sin_emb = np.asarray(sin_emb, f32)

    lam1 = np.exp(np.sum(np.asarray(lambda_q1, f32) * np.asarray(lambda_k1, f32),
                         dtype=f32))
    lam2 = np.exp(np.sum(np.asarray(lambda_q2, f32) * np.asarray(lambda_k2, f32),
                         dtype=f32))
    lam = np.float32(lam1 - lam2 + LAMBDA_INIT)

    # per-batch transposed activations (bf16)
    xt = {}
    for b in range(B):
        xt[("q", b)] = np.ascontiguousarray(q[b].T).astype(bf16)
        xt[("k", b)] = np.ascontiguousarray(k[b].T).astype(bf16)
        xt[("v", b)] = np.ascontiguousarray(v[b].T).astype(bf16)

    # shared constant tensors
    base_c = cos_emb[:S, :QKD]          # [S, 64]
    base_s = sin_emb[:S, :QKD]
    crep = np.ascontiguousarray(np.tile(base_c.T, (2, 1))).astype(bf16)  # [128, S]
    srep = np.ascontiguousarray(np.tile(base_s.T, (2, 1))).astype(bf16)
    pmat = np.zeros((128, 128), f32)
    for blk in range(2):
        o = blk * 64
        for i in range(QKD // 2):
            pmat[o + 2 * i, o + 2 * i + 1] = 1.0     # lhsT[2i, 2i+1]
            pmat[o + 2 * i + 1, o + 2 * i] = -1.0    # lhsT[2i+1, 2i]
    pmat = pmat.astype(bf16)
    # tri01[p, n] = 0 if p > n (key index > query index within the 128 block)
    tri01 = (np.triu(np.ones((128, 128), f32), 0)).astype(bf16)
    ones_a = np.ones((128, 128), bf16)
    o64 = np.full((128, 128), 1.0 / 64, bf16)
    neglam = np.full((128, 1), -lam, f32)

    per_core = []
    for c in range(NCORES):
        b, grp = c // GRPS, c % GRPS
        heads = [HPC * grp + j for j in range(HPC)]
        # wq/wk columns: tile h; partitions [64g, 64g+64) = map g of head h,
        # original feature order (interleaved pairs)
        cols = []
        for h in range(HPC):
            hg = heads[h]
            for g in range(2):
                cols.extend(hg * HD + g * QKD + d for d in range(QKD))
        cols = np.asarray(cols)
        vrows = np.asarray([h * HD + d for h in heads for d in range(HD)])
        wq_t = np.ascontiguousarray(Wq[cols, :].T).astype(bf16)
        wk_t = np.ascontiguousarray(Wk[cols, :].T).astype(bf16)
        wv_t = np.ascontiguousarray(Wv[vrows, :].T).astype(bf16)
        gtile = np.tile(gnorm_w, HPC)                       # [512]
        wo_t = np.ascontiguousarray(
            ((1.0 - LAMBDA_INIT) * Wo[:, vrows] * gtile[None, :]).T).astype(bf16)
        per_core.append({
            "xq_t": xt[("q", b)], "xk_t": xt[("k", b)], "xv_t": xt[("v", b)],
            "wq_t": wq_t, "wk_t": wk_t, "wv_t": wv_t, "wo_t": wo_t,
            "crep": crep, "srep": srep, "pmat": pmat,
            "tri01": tri01, "ones_a": ones_a, "o64": o64, "neglam": neglam,
        })
    return per_core


def _install_ntff_hook():
    """antenv.axon_hooks is absent in this image; synthesize it so
    run_bass_kernel_spmd(trace=True) can capture NTFF profiles."""
    import sys as _sys
    import types

    if "antenv.axon_hooks" in _sys.modules:
        return
    import antenv
    mod = types.ModuleType("antenv.axon_hooks")
    state = {"hook": None}
    mod.set_axon_ntff_profile_hook = lambda h: state.__setitem__("hook", h)
    mod.get_axon_ntff_profile_hook = lambda: state["hook"]
    _sys.modules["antenv.axon_hooks"] = mod
    antenv.axon_hooks = mod
    try:
        from trn_agent_boot.trn_boot import _ntff_profile_via_ctypes
        state["hook"] = _ntff_profile_via_ctypes("/opt/axon/libaxon_pjrt.so")
    except Exception as e:  # degrade: trace skipped, run still works
        print("ntff hook install failed:", e)


def kernel(q, k, v, Wq, Wk, Wv, Wo, lambda_q1, lambda_k1, lambda_q2,
           lambda_k2, gnorm_w, cos_emb, sin_emb, mask, _trace=False):
    if _trace:
        _install_ntff_hook()
    global _PROGRAM
    if _PROGRAM is None:
        _PROGRAM = _build_program()
    nc = _PROGRAM

    in_maps = _host_prep(q, k, v, Wq, Wk, Wv, Wo, lambda_q1, lambda_k1,
                         lambda_q2, lambda_k2, gnorm_w, cos_emb, sin_emb)

    from concourse.bass_utils import run_bass_kernel_spmd
    res = run_bass_kernel_spmd(nc, in_maps, core_ids=list(range(NCORES)),
                               trace=_trace)
    kernel.last_result = res

    y = np.zeros((B, S, HID), np.float32)
    for c in range(NCORES):
        y[c // GRPS] += np.asarray(res.results[c]["y_t"]).T.astype(np.float32)
    return y


# revision 55
# speedup vs baseline: 1.2188x; 1.0106x over previous
"""Multi-head differential attention on 8 Trainium2 NeuronCores.

Sharding: data-parallel over batch (B=2) x tensor-parallel over heads
(16 heads -> 4 per core). Core c handles batch c//4 and heads
4*(c%4) .. 4*(c%4)+3. Each core computes its heads' attention output and a
partial output projection; the host sums the 4 partials per batch.

v3: bf16 matmul operands, map-packed Q/K (concurrent K=64 score matmuls),
software-pipelined attention emission (PV/SM staggered 2 chunks behind
scores/exp so the in-order PE queue never waits on the scalar engine),
lazy weight DMAs so the first matmul isn't stuck behind the full DMA
trigger queue, batched RMS sqrt (one ACT table switch instead of 32),
rope interleaved into the projection phases.
"""

import math
import os
import sys

sys.path.insert(0, "/opt/trn_rl_repo")

import numpy as np

B, S, HID, NH = 2, 2048, 2048, 16
HD = HID // NH          # 128
QKD = HD // 2           # 64
NCORES = 8
GRPS = NCORES // B      # head groups per batch
HPC = NH // GRPS        # heads per core = 4
LAYER_ID = 1
LAMBDA_INIT = 0.8 - 0.6 * math.exp(-0.3 * LAYER_ID)
EPS = 1e-6

NB = S // 512           # 4 seq blocks of 512
NKC = S // 128          # 16 key chunks of 128

_PROGRAM = None         # compiled bass program, reused across calls


def _build_program():
    import concourse.bass as bass
    import concourse.tile as tile
    from concourse import bacc, mybir

    f32 = mybir.dt.float32
    f32r = mybir.dt.float32r
    bf16 = mybir.dt.bfloat16
    Alu = mybir.AluOpType
    Act = mybir.ActivationFunctionType

    nc = bacc.Bacc(None, target_bir_lowering=False, debug=False)

    def din(name, shape, dt=bf16):
        return nc.dram_tensor(name, shape, dt, kind="ExternalInput").ap()

    io = {
        "xq_t": din("xq_t", [HID, S]),
        "xk_t": din("xk_t", [HID, S]),
        "xv_t": din("xv_t", [HID, S]),
        "wq_t": din("wq_t", [HID, 512]),
        "wk_t": din("wk_t", [HID, 512]),
        "wv_t": din("wv_t", [HID, 512]),
        "wo_t": din("wo_t", [512, HID]),
        "crep": din("crep", [128, S]),
        "srep": din("srep", [128, S]),
        "pmat": din("pmat", [128, 128]),
        "tri01": din("tri01", [128, 128]),
        "ones_a": din("ones_a", [128, 128]),
        "o64": din("o64", [128, 128]),
        "neglam": din("neglam", [128, 1], f32),
    }
    y_t = nc.dram_tensor("y_t", [HID, S], bf16, kind="ExternalOutput").ap()

    from contextlib import ExitStack

    with tile.TileContext(nc) as tc, ExitStack() as ctx:
        persist = ctx.enter_context(tc.tile_pool(name="persist", bufs=1))
        constp = ctx.enter_context(tc.tile_pool(name="constp", bufs=1))

        # constants
        crep = constp.tile([128, S], bf16, name="crep_sb", tag="crep")
        srep = constp.tile([128, S], bf16, name="srep_sb", tag="srep")
        pmat = constp.tile([128, 128], bf16, name="pmat_sb", tag="pmat")
        tri01 = constp.tile([128, 128], bf16, name="tri01_sb", tag="tri01")
        ones_a = constp.tile([128, 128], bf16, name="ones_a_sb", tag="ones_a")
        o64 = constp.tile([128, 128], bf16, name="o64_sb", tag="o64")
        neglam = constp.tile([128, 1], f32, name="neglam_sb", tag="neglam")
        epsb = constp.tile([128, 1], f32, name="epsb", tag="epsb")
        nc.vector.memset(epsb[:], EPS)

        def emit_const_dmas():
            # deferred onto the scalar engine's DGE queue (behind the wq
            # chunks): keeps 1MB of table transfers out of the sync queue so
            # neither the first matmul nor the x-chunk stream waits on them
            for t, key in ((crep, "crep"), (srep, "srep"), (pmat, "pmat"),
                           (tri01, "tri01"), (ones_a, "ones_a"),
                           (o64, "o64"), (neglam, "neglam")):
                nc.scalar.dma_start(out=t[:], in_=io[key][:])

        # persistent tensors, per head h: Q^T/K^T [128, S] bf16 where
        # partitions [64g, 64g+64) hold softmax-map g's features.
        QT = [persist.tile([128, S], bf16, name=f"qt{h}", tag=f"qt{h}")
              for h in range(HPC)]
        KT = [persist.tile([128, S], bf16, name=f"kt{h}", tag=f"kt{h}")
              for h in range(HPC)]
        # V natural layout per 128-seq chunk: [128 seq, 4 heads * 128 feat]
        VH = [persist.tile([128, 512], bf16, name=f"vh{s}", tag=f"vh{s}")
              for s in range(NKC)]
        # combined attention output, transposed: [feat, seq]
        U = [persist.tile([128, S], bf16, name=f"u{h}", tag=f"u{h}")
             for h in range(HPC)]
        # output projection weights (prefetched early)
        wo = [persist.tile([128, S], bf16, name=f"wo{h}", tag=f"wo{h}")
              for h in range(HPC)]

        # ------------- phase P + R: projections with rope interleaved -------
        # rope: y = x*cos + (P x)*sin, P = signed pair-swap (PE matmul).
        rope_q = []   # pending (tiles, n) rope tasks

        with tc.tile_pool(name="wp", bufs=1) as wp, \
             tc.tile_pool(name="xp", bufs=6) as xp, \
             tc.tile_pool(name="rtp", bufs=4) as rtp, \
             tc.tile_pool(name="pp", bufs=1, space="PSUM") as pp:

            def emit_rope():
                T4, n = rope_q.pop(0)
                sl = slice(n * 512, (n + 1) * 512)
                pxs = []
                for h in range(HPC):
                    px = pp.tile([128, 512], f32, name=f"px_{T4[h].name}_{n}",
                                 tag="px", bufs=3)
                    nc.tensor.matmul(px[:], pmat[:], T4[h][:, sl],
                                     start=True, stop=True)
                    pxs.append(px)
                for h in range(HPC):
                    tmp = rtp.tile([128, 512], bf16,
                                   name=f"rs_{T4[h].name}_{n}", tag="rs")
                    nc.vector.tensor_mul(tmp[:], pxs[h][:], srep[:, sl])
                    tmp2 = rtp.tile([128, 512], bf16,
                                    name=f"rc_{T4[h].name}_{n}", tag="rc")
                    nc.vector.tensor_mul(tmp2[:], T4[h][:, sl], crep[:, sl])
                    nc.vector.tensor_add(T4[h][:, sl], tmp[:], tmp2[:])

            wnames = ("wq_t", "wk_t", "wv_t")
            wts = {m: [wp.tile([128, 512], bf16, name=f"{wnames[m]}_{kc}",
                               tag=f"w{m}_{kc}") for kc in range(NKC)]
                   for m in range(3)}

            def wdma(m, kc):
                # weight DMAs ride the scalar engine's DGE queue so they
                # never contend with the x-chunk triggers on sync
                nc.scalar.dma_start(out=wts[m][kc][:],
                                    in_=io[wnames[m]][kc * 128:(kc + 1) * 128, :])

            for mode, xname in enumerate(("xq_t", "xk_t", "xv_t")):
                wt = wts[mode]
                xin = io[xname]
                for n in range(NB):
                    blk_idx = mode * NB + n
                    ps = [pp.tile([128, 512], f32, name=f"pp{t}_{mode}_{n}",
                                  tag="pp", bufs=5) for t in range(4)]
                    for kc in range(NKC):
                        if mode == 0 and n == 0:
                            wdma(0, kc)       # lazy: wq arrives just ahead
                        xck = xp.tile([128, 512], bf16, name=f"x_{mode}_{n}_{kc}",
                                      tag="x")
                        nc.sync.dma_start(
                            out=xck[:],
                            in_=xin[kc * 128:(kc + 1) * 128,
                                    n * 512:(n + 1) * 512])
                        if mode == 0 and n == 1:
                            wdma(1, kc)       # wk during q-proj
                        elif mode == 0 and n == 2:
                            wdma(2, kc)       # wv during q-proj
                        elif mode == 0 and n == 3 and kc % 4 == 0:
                            h = kc // 4       # wo prefetch, needed in phase Y
                            nc.scalar.dma_start(
                                out=wo[h][:],
                                in_=io["wo_t"][h * 128:(h + 1) * 128, :])
                        for t in range(4):
                            if mode < 2:
                                lhsT = wt[kc][:, t * 128:(t + 1) * 128]
                                rhs = xck[:]
                            else:
                                lhsT = xck[:, t * 128:(t + 1) * 128]
                                rhs = wt[kc][:]
                            nc.tensor.matmul(ps[t][:], lhsT, rhs,
                                             start=(kc == 0), stop=(kc == NKC - 1))
                    # drains on the scalar engine (vector is busy with rope)
                    for t in range(4):
                        if mode == 0:
                            nc.scalar.copy(QT[t][:, n * 512:(n + 1) * 512],
                                           ps[t][:])
                        elif mode == 1:
                            nc.scalar.copy(KT[t][:, n * 512:(n + 1) * 512],
                                           ps[t][:])
                        else:
                            nc.scalar.copy(VH[n * 4 + t][:], ps[t][:])
                    if blk_idx == 0:
                        emit_const_dmas()
                    if mode == 0:
                        rope_q.append((QT, n))
                    elif mode == 1:
                        rope_q.append((KT, n))
                    if blk_idx >= 2 and rope_q and blk_idx - (
                            rope_q[0][1] + (0 if rope_q[0][0] is QT else NB)) >= 2:
                        emit_rope()
            while rope_q:
                emit_rope()

        # ------------- phase A: attention, PV/SM staggered 2 chunks ---------
        with tc.tile_pool(name="spp", bufs=1, space="PSUM") as spp, \
             tc.tile_pool(name="pvp", bufs=1, space="PSUM") as pvp, \
             tc.tile_pool(name="smp", bufs=1, space="PSUM") as smp, \
             tc.tile_pool(name="ep", bufs=5) as ep, \
             tc.tile_pool(name="cb", bufs=2) as cb:
            for h in range(HPC):
                for qb in range(NB):
                    pv = pvp.tile([128, 1024], f32, name=f"pv_{h}_{qb}",
                                  tag="pv")
                    sm = smp.tile([128, 1024], f32, name=f"sm_{h}_{qb}",
                                  tag="sm")
                    nkc = 4 * qb + 4
                    pvsm_pending = []

                    def make_pvsm(kc, E, qoff, first, last, h=h, pv=pv, sm=sm):
                        def emit():
                            for g in (0, 1):
                                nc.tensor.matmul(
                                    pv[:, g * 512 + qoff:g * 512 + 512],
                                    VH[kc][:, h * 128:(h + 1) * 128],
                                    E[:, g * 512 + qoff:g * 512 + 512],
                                    start=first, stop=last)
                            for g in (0, 1):
                                nc.tensor.matmul(
                                    sm[:, g * 512 + qoff:g * 512 + 512],
                                    ones_a[:],
                                    E[:, g * 512 + qoff:g * 512 + 512],
                                    start=first, stop=last)
                        return emit

                    for kc in range(nkc):
                        j = kc - 4 * qb  # >= 0 on the causal diagonal band
                        qoff = j * 128 if j >= 0 else 0
                        sp = spp.tile([128, 1024], f32, name=f"s_{h}_{qb}_{kc}",
                                      tag="sp", bufs=2)
                        for g in (0, 1):
                            goff = g * 512
                            nc.tensor.matmul(
                                sp[:, goff + qoff:goff + 512],
                                KT[h][g * 64:(g + 1) * 64,
                                      kc * 128:(kc + 1) * 128],
                                QT[h][g * 64:(g + 1) * 64,
                                      qb * 512 + qoff:(qb + 1) * 512],
                                start=True, stop=True)
                        E = ep.tile([128, 1024], bf16, name=f"e_{h}_{qb}_{kc}",
                                    tag="e")
                        if qoff == 0:
                            nc.scalar.activation(E[:], sp[:], Act.Exp,
                                                 scale=0.125)
                        else:
                            for g in (0, 1):
                                nc.scalar.activation(
                                    E[:, g * 512 + qoff:g * 512 + 512],
                                    sp[:, g * 512 + qoff:g * 512 + 512],
                                    Act.Exp, scale=0.125)
                        if j >= 0:
                            # causal mask inside the diagonal 128x128 block,
                            # applied multiplicatively on the vector engine
                            for g in (0, 1):
                                esl = E[:, g * 512 + qoff:g * 512 + qoff + 128]
                                nc.vector.tensor_mul(esl, esl, tri01[:])
                        pvsm_pending.append(make_pvsm(
                            kc, E, qoff, kc == 0, kc == nkc - 1))
                        if len(pvsm_pending) > 3:
                            pvsm_pending.pop(0)()
                    while pvsm_pending:
                        pvsm_pending.pop(0)()

                    # combine: U = pv0/sm0 - lam * pv1/sm1
                    usl = U[h][:, qb * 512:(qb + 1) * 512]
                    rb = cb.tile([128, 1024], f32, name=f"rb_{h}_{qb}", tag="rb")
                    nc.vector.reciprocal_approx_fast(rb[:], sm[:])
                    t1 = cb.tile([128, 512], f32, name=f"t1_{h}_{qb}", tag="t1")
                    t2 = cb.tile([128, 512], f32, name=f"t2_{h}_{qb}", tag="t2")
                    nc.vector.tensor_mul(t1[:], pv[:, 0:512], rb[:, 0:512])
                    nc.vector.tensor_mul(t2[:], pv[:, 512:1024], rb[:, 512:1024])
                    nc.vector.scalar_tensor_tensor(
                        usl, t2[:], neglam[:], t1[:],
                        op0=Alu.mult, op1=Alu.add)

        # ------------- phase N + Y: batched RMS norm, output projection -----
        with tc.tile_pool(name="yp", bufs=4, space="PSUM") as yp, \
             tc.tile_pool(name="sqp", bufs=4, space="PSUM") as sqp, \
             tc.tile_pool(name="ys", bufs=4) as ys, \
             tc.tile_pool(name="nb", bufs=4) as nb:
            def emit_norm(qc):
                for h in range(HPC):
                    usl = U[h][:, qc * 512:(qc + 1) * 512]
                    sq = nb.tile([128, 512], bf16, name=f"sq_{h}_{qc}", tag="sq")
                    nc.vector.tensor_mul(sq[:], usl, usl)
                    ssq = sqp.tile([128, 512], f32, name=f"ssq_{h}_{qc}",
                                   tag="ssq")
                    nc.tensor.matmul(ssq[:], ones_a[:], sq[:],
                                     start=True, stop=True)
                    sd = nb.tile([128, 512], f32, name=f"sd_{h}_{qc}", tag="sd")
                    nc.scalar.activation(sd[:], ssq[:], Act.Sqrt,
                                         scale=1.0 / HD, bias=epsb[:])
                    rstd = nb.tile([128, 512], f32, name=f"rstd_{h}_{qc}",
                                   tag="rstd")
                    nc.vector.reciprocal_approx_fast(rstd[:], sd[:])
                    nc.vector.tensor_mul(usl, usl, rstd[:])

            emit_norm(0)
            for qc in range(NB):
                # normalize for qc+1 a third of the way into Y(qc): late
                # enough that Y(qc)'s first psum-recycling drains aren't
                # queued behind the norm chains, early enough that U(qc+1)
                # is ready before Y(qc+1) starts
                for oc in range(NKC):
                    if oc == 5 and qc + 1 < NB:
                        emit_norm(qc + 1)
                    py = yp.tile([128, 512], f32, name=f"py_{oc}_{qc}", tag="py")
                    for h in range(HPC):
                        nc.tensor.matmul(
                            py[:],
                            wo[h][:, oc * 128:(oc + 1) * 128],
                            U[h][:, qc * 512:(qc + 1) * 512],
                            start=(h == 0), stop=(h == HPC - 1))
                    yst = ys.tile([128, 512], bf16, name=f"yst_{oc}_{qc}",
                                  tag="yst")
                    if (oc + qc) % 2 == 0:
                        nc.vector.tensor_copy(yst[:], py[:])
                        eng = nc.sync
                    else:
                        nc.scalar.copy(yst[:], py[:])
                        eng = nc.scalar
                    # alternate DGE queues so the output triggers never back
                    # up behind one engine's serial trigger stream
                    eng.dma_start(
                        out=y_t[oc * 128:(oc + 1) * 128, qc * 512:(qc + 1) * 512],
                        in_=yst[:])

    nc.compile()
    return nc


def _host_prep(q, k, v, Wq, Wk, Wv, Wo, lambda_q1, lambda_k1, lambda_q2,
               lambda_k2, gnorm_w, cos_emb, sin_emb):
    import ml_dtypes

    f32 = np.float32
    bf16 = ml_dtypes.bfloat16
    q = np.asarray(q, f32); k = np.asarray(k, f32); v = np.asarray(v, f32)
    Wq = np.asarray(Wq, f32); Wk = np.asarray(Wk, f32)
    Wv = np.asarray(Wv, f32); Wo = np.asarray(Wo, f32)
    gnorm_w = np.asarray(gnorm_w, f32)
    cos_emb = np.asarray(cos_emb, f32); sin_emb = np.asarray(sin_emb, f32)

    lam1 = np.exp(np.sum(np.asarray(lambda_q1, f32) * np.asarray(lambda_k1, f32),
                         dtype=f32))
    lam2 = np.exp(np.sum(np.asarray(lambda_q2, f32) * np.asarray(lambda_k2, f32),
                         dtype=f32))
    lam = np.float32(lam1 - lam2 + LAMBDA_INIT)

    # per-batch transposed activations (bf16)
    xt = {}
    for b in range(B):
        xt[("q", b)] = np.ascontiguousarray(q[b].T).astype(bf16)
        xt[("k", b)] = np.ascontiguousarray(k[b].T).astype(bf16)
        xt[("v", b)] = np.ascontiguousarray(v[b].T).astype(bf16)

    # shared constant tensors
    base_c = cos_emb[:S, :QKD]          # [S, 64]
    base_s = sin_emb[:S, :QKD]
    crep = np.ascontiguousarray(np.tile(base_c.T, (2, 1))).astype(bf16)  # [128, S]
    srep = np.ascontiguousarray(np.tile(base_s.T, (2, 1))).astype(bf16)
    pmat = np.zeros((128, 128), f32)
    for blk in range(2):
        o = blk * 64
        for i in range(QKD // 2):
            pmat[o + 2 * i, o + 2 * i + 1] = 1.0     # lhsT[2i, 2i+1]
            pmat[o + 2 * i + 1, o + 2 * i] = -1.0    # lhsT[2i+1, 2i]
    pmat = pmat.astype(bf16)
    # tri01[p, n] = 0 if p > n (key index > query index within the 128 block)
    tri01 = (np.triu(np.ones((128, 128), f32), 0)).astype(bf16)
    ones_a = np.ones((128, 128), bf16)
    o64 = np.full((128, 128), 1.0 / 64, bf16)
    neglam = np.full((128, 1), -lam, f32)

    per_core = []
    for c in range(NCORES):
        b, grp = c // GRPS, c % GRPS
        heads = [HPC * grp + j for j in range(HPC)]
        # wq/wk columns: tile h; partitions [64g, 64g+64) = map g of head h,
        # original feature order (interleaved pairs)
        cols = []
        for h in range(HPC):
            hg = heads[h]
            for g in range(2):
                cols.extend(hg * HD + g * QKD + d for d in range(QKD))
        cols = np.asarray(cols)
        vrows = np.asarray([h * HD + d for h in heads for d in range(HD)])
        wq_t = np.ascontiguousarray(Wq[cols, :].T).astype(bf16)
        wk_t = np.ascontiguousarray(Wk[cols, :].T).astype(bf16)
        wv_t = np.ascontiguousarray(Wv[vrows, :].T).astype(bf16)
        gtile = np.tile(gnorm_w, HPC)                       # [512]
        wo_t = np.ascontiguousarray(
            ((1.0 - LAMBDA_INIT) * Wo[:, vrows] * gtile[None, :]).T).astype(bf16)
        per_core.append({
            "xq_t": xt[("q", b)], "xk_t": xt[("k", b)], "xv_t": xt[("v", b)],
            "wq_t": wq_t, "wk_t": wk_t, "wv_t": wv_t, "wo_t": wo_t,
            "crep": crep, "srep": srep, "pmat": pmat,
            "tri01": tri01, "ones_a": ones_a, "o64": o64, "neglam": neglam,
        })
    return per_core


def _install_ntff_hook():
    """antenv.axon_hooks is absent in this image; synthesize it so
    run_bass_kernel_spmd(trace=True) can capture NTFF profiles."""
    import sys as _sys
    import types

    if "antenv.axon_hooks" in _sys.modules:
        return
    import antenv
    mod = types.ModuleType("antenv.axon_hooks")
    state = {"hook": None}
    mod.set_axon_ntff_profile_hook = lambda h: state.__setitem__("hook", h)
    mod.get_axon_ntff_profile_hook = lambda: state["hook"]
    _sys.modules["antenv.axon_hooks"] = mod
    antenv.axon_hooks = mod
    try:
        from trn_agent_boot.trn_boot import _ntff_profile_via_ctypes
        state["hook"] = _ntff_profile_via_ctypes("/opt/axon/libaxon_pjrt.so")
    except Exception as e:  # degrade: trace skipped, run still works
        print("ntff hook install failed:", e)


def kernel(q, k, v, Wq, Wk, Wv, Wo, lambda_q1, lambda_k1, lambda_q2,
           lambda_k2, gnorm_w, cos_emb, sin_emb, mask, _trace=False):
    if _trace:
        _install_ntff_hook()
    global _PROGRAM
    if _PROGRAM is None:
        _PROGRAM = _build_program()
    nc = _PROGRAM

    in_maps = _host_prep(q, k, v, Wq, Wk, Wv, Wo, lambda_q1, lambda_k1,
                         lambda_q2, lambda_k2, gnorm_w, cos_emb, sin_emb)

    from concourse.bass_utils import run_bass_kernel_spmd
    res = run_bass_kernel_spmd(nc, in_maps, core_ids=list(range(NCORES)),
                               trace=_trace)
    kernel.last_result = res

    y = np.zeros((B, S, HID), np.float32)
    for c in range(NCORES):
        y[c // GRPS] += np.asarray(res.results[c]["y_t"]).T.astype(np.float32)
    return y
